# revision 2
# baseline (speedup 1.0000x reference)
"""Trainium2 Bass kernel for nn_Attention_60567628808865.

Dense transformer attention block (B=4, S=1024, H=4096, NH=32, D=128):
  qkv = x @ W_pack; RoPE(q, k); causal-masked softmax attention; out @ W_o.

Sharding: tensor-parallel over heads across 8 NeuronCores. Each core computes
4 heads end-to-end (QKV projection with its W_pack column slice, attention,
and its W_o row-slice partial of the output projection); the host sums the 8
partial outputs.

All matmuls run in float32r (TF32) at full PE rate; accumulation is fp32 in
PSUM. Everything on-chip works in a transposed layout (features on the
partition axis) so no transposes are needed anywhere:
  qT/kT [d, t] <- lhsT=W_qk, rhs=xT      scoresT [tk, tq] <- lhsT=kT, rhs=qT
  v [t, d]     <- lhsT=xT,   rhs=W_v     attnT [d, tq]    <- lhsT=v,  rhs=expT
  out [t, f]   <- lhsT=attnT, rhs=W_o
Softmax runs unnormalized (no max-subtraction; scores are O(1) by
construction and exp(-1e9)=0), with the denominator computed by a ones-vector
matmul accumulated in PSUM and applied after PV via a K=1 broadcast matmul.
RoPE's rotate-half is a partition shift, done for free in the DMA that loads
q/k back from scratch, with the sign folded into the host-built sin table.
"""
import numpy as np

import concourse.bass as bass  # noqa: F401  (AP types come via tile/bacc)
import concourse.tile as tile
from contextlib import ExitStack
from concourse import bacc, mybir
from concourse import bass_utils

F32 = mybir.dt.float32
F32R = mybir.dt.float32r
AF = mybir.ActivationFunctionType
ALU = mybir.AluOpType

B, S, H, NH = 4, 1024, 4096, 32
D = H // NH          # 128
T = B * S            # 4096 tokens
N_CORES = 8
HC = NH // N_CORES   # 4 heads per core
SCALE = float(1.0 / np.sqrt(D))
ROPE_BASE = 10000.0

TB = 256             # phase-1 token block (moving dim; >=256 keeps f32r at full rate)
NTB = T // TB        # 16
KT = H // 128        # 32 contraction tiles

_CACHE = {}


def _build_module():
    nc = bacc.Bacc("TRN2", target_bir_lowering=False, debug=False,
                   num_devices=N_CORES)

    xT = nc.dram_tensor("xT", [H, T], F32R, kind="ExternalInput").ap()
    wqk = nc.dram_tensor("wqk", [H, 2 * HC * D], F32R, kind="ExternalInput").ap()
    wv = nc.dram_tensor("wv", [H, HC * D], F32R, kind="ExternalInput").ap()
    wo = nc.dram_tensor("wo", [HC * D, H], F32R, kind="ExternalInput").ap()
    cosT = nc.dram_tensor("cosT", [D, T], F32, kind="ExternalInput").ap()
    sinS = nc.dram_tensor("sinS", [D, T], F32, kind="ExternalInput").ap()
    maskT = nc.dram_tensor("maskT", [B, S, S], mybir.dt.bfloat16, kind="ExternalInput").ap()
    out_p = nc.dram_tensor("out_p", [T, H], F32, kind="ExternalOutput").ap()
    ones128 = nc.inline_tensor(np.ones((128, 1), np.float32), "ones128").ap().bitcast(F32R)
    ones1 = nc.inline_tensor(np.ones((1, 128), np.float32), "ones1").ap().bitcast(F32R)

    with tile.TileContext(nc) as tc, \
         nc.allow_low_precision(reason="tf32 matmuls; verified against reference"):
        with ExitStack() as octx:
            dram = octx.enter_context(tc.tile_pool(name="dram", bufs=1, space="DRAM"))
            cpool = octx.enter_context(tc.tile_pool(name="consts", bufs=1))
            # scratch: qkT rows (pass p, m): [q_2p, k_2p, q_2p+1, k_2p+1]
            qkT_d = dram.tile([2 * HC * D, T], F32R)
            v_d = dram.tile([T, HC * D], F32R)

            o128 = cpool.tile([128, 1], F32R)
            nc.sync.dma_start(o128[:], ones128[:])
            o1 = cpool.tile([1, 128], F32R)
            nc.sync.dma_start(o1[:], ones1[:])

            # ---------------- Phase 1: QKV projection ----------------
            with ExitStack() as ctx:
                wpool = ctx.enter_context(tc.tile_pool(name="p1w", bufs=KT))
                xpool = ctx.enter_context(tc.tile_pool(name="p1x", bufs=2 * KT))
                opool = ctx.enter_context(tc.tile_pool(name="p1o", bufs=8))
                pqk = ctx.enter_context(tc.tile_pool(name="p1pqk", bufs=4, space="PSUM"))
                pv = ctx.enter_context(tc.tile_pool(name="p1pv", bufs=2, space="PSUM"))

                for p in range(2):
                    wqk_t = []
                    wv_t = []
                    for kk in range(KT):
                        wt = wpool.tile([128, 512], F32R, tag="wqk")
                        nc.sync.dma_start(
                            wt[:], wqk[kk * 128:(kk + 1) * 128, p * 512:(p + 1) * 512])
                        wqk_t.append(wt)
                        vt_ = wpool.tile([128, 256], F32R, tag="wv")
                        nc.sync.dma_start(
                            vt_[:], wv[kk * 128:(kk + 1) * 128, p * 256:(p + 1) * 256])
                        wv_t.append(vt_)

                    for tb in range(NTB):
                        t0 = tb * TB
                        xt = []
                        for kk in range(KT):
                            x_ = xpool.tile([128, TB], F32R, tag="x")
                            nc.sync.dma_start(
                                x_[:], xT[kk * 128:(kk + 1) * 128, t0:t0 + TB])
                            xt.append(x_)
                        for m in range(4):
                            ps = pqk.tile([128, TB], F32, tag="qk")
                            for kk in range(KT):
                                nc.tensor.matmul(
                                    ps[:], wqk_t[kk][:, m * 128:(m + 1) * 128],
                                    xt[kk][:], start=(kk == 0), stop=(kk == KT - 1))
                            qs = opool.tile([128, TB], F32R, tag="qko")
                            nc.vector.tensor_copy(qs[:], ps[:])
                            nc.sync.dma_start(
                                qkT_d[(p * 4 + m) * 128:(p * 4 + m + 1) * 128,
                                      t0:t0 + TB], qs[:])
                        for mt in range(2):
                            ps = pv.tile([128, 256], F32, tag="v")
                            for kk in range(KT):
                                nc.tensor.matmul(
                                    ps[:], xt[kk][:, mt * 128:(mt + 1) * 128],
                                    wv_t[kk][:], start=(kk == 0), stop=(kk == KT - 1))
                            vs = opool.tile([128, 256], F32R, tag="vo")
                            nc.vector.tensor_copy(vs[:], ps[:])
                            nc.sync.dma_start(
                                v_d[t0 + mt * 128:t0 + (mt + 1) * 128,
                                    p * 256:(p + 1) * 256], vs[:])

            # ---------------- Phase 2+3: attention + W_o ----------------
            with ExitStack() as ctx:
                wopool = ctx.enter_context(tc.tile_pool(name="p2wo", bufs=4))
                mpool = ctx.enter_context(tc.tile_pool(name="p2m", bufs=1))
                tpool = ctx.enter_context(tc.tile_pool(name="p2t", bufs=2))
                tmpool = ctx.enter_context(tc.tile_pool(name="p2tm", bufs=1))
                epool = ctx.enter_context(tc.tile_pool(name="p2e", bufs=4))
                apool = ctx.enter_context(tc.tile_pool(name="p2a", bufs=1))
                opool = ctx.enter_context(tc.tile_pool(name="p2o", bufs=4))
                ps_s = ctx.enter_context(tc.tile_pool(name="p2ps", bufs=2, space="PSUM"))
                ps_d = ctx.enter_context(tc.tile_pool(name="p2pd", bufs=1, space="PSUM"))
                ps_av = ctx.enter_context(tc.tile_pool(name="p2pav", bufs=2, space="PSUM"))
                ps_bc = ctx.enter_context(tc.tile_pool(name="p2pbc", bufs=1, space="PSUM"))
                ps_o = ctx.enter_context(tc.tile_pool(name="p2po", bufs=2, space="PSUM"))

                wo_t = []
                for l in range(HC):
                    wt = wopool.tile([128, H], F32R, tag="wo")
                    nc.sync.dma_start(wt[:], wo[l * 128:(l + 1) * 128, :])
                    wo_t.append(wt)

                for b in range(B):
                    bs = b * S
                    mask_b = mpool.tile([128, 8 * S], mybir.dt.bfloat16, tag="mask")
                    nc.sync.dma_start(
                        mask_b[:].rearrange("p (mt t) -> p mt t", mt=8),
                        maskT[b].rearrange("(mt p) t -> p mt t", p=128))
                    cos_b = mpool.tile([128, S], F32, tag="cos")
                    nc.sync.dma_start(cos_b[:], cosT[:, bs:bs + S])
                    sin_b = mpool.tile([128, S], F32, tag="sin")
                    nc.sync.dma_start(sin_b[:], sinS[:, bs:bs + S])

                    attn_t = []
                    for l in range(HC):
                        rq = (4 * (l // 2) + 2 * (l % 2)) * 128
                        rk = rq + 128
                        vcol = (l // 2) * 256 + (l % 2) * 128

                        def load_rope(row, rope_tag):
                            raw = tpool.tile([128, S], F32R, tag="rraw")
                            nc.sync.dma_start(raw[:], qkT_d[row:row + 128, bs:bs + S])
                            rot = tpool.tile([128, S], F32R, tag="rrot")
                            nc.sync.dma_start(rot[0:64, :],
                                              qkT_d[row + 64:row + 128, bs:bs + S])
                            nc.sync.dma_start(rot[64:128, :],
                                              qkT_d[row:row + 64, bs:bs + S])
                            m1 = tmpool.tile([128, S], F32, tag="m1")
                            nc.vector.tensor_tensor(m1[:], raw[:].bitcast(F32),
                                                    cos_b[:], op=ALU.mult)
                            m2 = tmpool.tile([128, S], F32, tag="m2")
                            nc.vector.tensor_tensor(m2[:], rot[:].bitcast(F32),
                                                    sin_b[:], op=ALU.mult)
                            out_t = tpool.tile([128, S], F32R, tag=rope_tag)
                            nc.vector.tensor_tensor(out_t[:], m1[:], m2[:], op=ALU.add)
                            return out_t

                        q_rope = load_rope(rq, "qrope")
                        k_rope = load_rope(rk, "krope")
                        vt_ = tpool.tile([128, 8 * 128], F32R, tag="vt")
                        nc.sync.dma_start(
                            vt_[:].rearrange("p (kt d) -> p kt d", kt=8),
                            v_d[bs:bs + S, vcol:vcol + 128]
                               .rearrange("(kt p) d -> p kt d", p=128))

                        at = apool.tile([128, S], F32R, tag=f"attn{l}")
                        for nt in range(2):
                            nq = nt * 512
                            psd = ps_d.tile([1, 512], F32, tag="d")
                            psav = ps_av.tile([128, 512], F32, tag="av")
                            for mt in range(8):
                                pss = ps_s.tile([128, 512], F32, tag="s")
                                nc.tensor.matmul(
                                    pss[:], k_rope[:, mt * 128:(mt + 1) * 128],
                                    q_rope[:, nq:nq + 512], start=True, stop=True)
                                es = epool.tile([128, 512], F32, tag="es")
                                nc.vector.scalar_tensor_tensor(
                                    es[:], pss[:], SCALE,
                                    mask_b[:, mt * S + nq:mt * S + nq + 512],
                                    op0=ALU.mult, op1=ALU.add)
                                ef = epool.tile([128, 512], F32R, tag="ef")
                                nc.scalar.activation(ef[:], es[:], AF.Exp)
                                nc.tensor.matmul(psd[:], o128[:], ef[:],
                                                 start=(mt == 0), stop=(mt == 7))
                                nc.tensor.matmul(
                                    psav[:], vt_[:, mt * 128:(mt + 1) * 128], ef[:],
                                    start=(mt == 0), stop=(mt == 7))
                            rd = epool.tile([1, 512], F32R, tag="rd")
                            nc.vector.reciprocal(rd[:], psd[:])
                            psbc = ps_bc.tile([128, 512], F32, tag="bc")
                            nc.tensor.matmul(psbc[:], o1[:], rd[:], start=True, stop=True)
                            bcs = epool.tile([128, 512], F32, tag="bcs")
                            nc.vector.tensor_copy(bcs[:], psbc[:])
                            nc.vector.tensor_tensor(at[:, nq:nq + 512], psav[:],
                                                    bcs[:], op=ALU.mult)
                        attn_t.append(at)

                    for m in range(8):
                        for n in range(8):
                            pso = ps_o.tile([128, 512], F32, tag="o")
                            for l in range(HC):
                                nc.tensor.matmul(
                                    pso[:], attn_t[l][:, m * 128:(m + 1) * 128],
                                    wo_t[l][:, n * 512:(n + 1) * 512],
                                    start=(l == 0), stop=(l == HC - 1))
                            os_ = opool.tile([128, 512], F32, tag="os")
                            nc.vector.tensor_copy(os_[:], pso[:])
                            nc.sync.dma_start(
                                out_p[bs + m * 128:bs + (m + 1) * 128,
                                      n * 512:(n + 1) * 512], os_[:])
    nc.compile()
    return nc


def _host_prep(hidden_states, W_pack, W_o, attention_mask, position_ids):
    hidden_states = np.asarray(hidden_states, dtype=np.float32)
    W_pack = np.asarray(W_pack, dtype=np.float32)
    W_o = np.asarray(W_o, dtype=np.float32)
    attention_mask = np.asarray(attention_mask, dtype=np.float32)
    pos = np.asarray(position_ids)

    xT = np.ascontiguousarray(hidden_states.reshape(T, H).T)
    import ml_dtypes
    maskT = np.ascontiguousarray(
        attention_mask[:, 0].transpose(0, 2, 1).astype(ml_dtypes.bfloat16))

    inv = (1.0 / (ROPE_BASE ** (np.arange(0, D, 2, dtype=np.float64) / D)))
    inv = np.concatenate([inv, inv])                       # [D]
    ang = pos.astype(np.float64).reshape(T)[None, :] * inv[:, None]   # [D, T]
    cosT = np.cos(ang).astype(np.float32)
    sinT = np.sin(ang).astype(np.float32)
    sinS = sinT.copy()
    sinS[:64] = -sinT[:64]
    cosT = np.ascontiguousarray(cosT)
    sinS = np.ascontiguousarray(sinS)

    in_maps = []
    for c in range(N_CORES):
        h0 = c * HC
        # wqk column order per pass p: [q_{2p}, k_{2p}, q_{2p+1}, k_{2p+1}]
        qcols = [W_pack[:, (h0 + l) * D:(h0 + l + 1) * D] for l in range(HC)]
        kcols = [W_pack[:, H + (h0 + l) * D:H + (h0 + l + 1) * D] for l in range(HC)]
        vcols = [W_pack[:, 2 * H + (h0 + l) * D:2 * H + (h0 + l + 1) * D]
                 for l in range(HC)]
        wqk = np.ascontiguousarray(np.concatenate(
            [qcols[0], kcols[0], qcols[1], kcols[1],
             qcols[2], kcols[2], qcols[3], kcols[3]], axis=1))
        wv = np.ascontiguousarray(np.concatenate(vcols, axis=1))
        wo = np.ascontiguousarray(W_o[h0 * D:(h0 + HC) * D, :])
        in_maps.append({
            "xT": xT, "wqk": wqk, "wv": wv, "wo": wo,
            "cosT": cosT, "sinS": sinS, "maskT": maskT,
        })
    return in_maps


def kernel(hidden_states, W_pack, W_o, attention_mask, position_ids):
    if "nc" not in _CACHE:
        _CACHE["nc"] = _build_module()
    nc = _CACHE["nc"]
    in_maps = _host_prep(hidden_states, W_pack, W_o, attention_mask, position_ids)
    res = bass_utils.run_bass_kernel_spmd(nc, in_maps, core_ids=list(range(N_CORES)))
    out = res.results[0]["out_p"].astype(np.float64)
    for c in range(1, N_CORES):
        out += res.results[c]["out_p"]
    return out.reshape(B, S, H).astype(np.float32)


# revision 9
# speedup vs baseline: 1.1390x; 1.1390x over previous
"""Trainium2 Bass kernel for nn_Attention_60567628808865.

Dense transformer attention block (B=4, S=1024, H=4096, NH=32, D=128):
  qkv = x @ W_pack; RoPE(q, k); causal-masked softmax attention; out @ W_o.

Sharding: tensor-parallel over heads across 8 NeuronCores. Each core computes
4 heads end-to-end (QKV projection with its W_pack column slice, attention,
and its W_o row-slice partial of the output projection); the host sums the 8
partial outputs.

All matmuls run in float32r (TF32) at full PE rate; accumulation is fp32 in
PSUM. Everything on-chip works in a transposed layout (features on the
partition axis) so no transposes are needed anywhere:
  qT/kT [d, t] <- lhsT=W_qk, rhs=xT      scoresT [tk, tq] <- lhsT=kT, rhs=qT
  v [t, d]     <- lhsT=xT,   rhs=W_v     attnT [d, tq]    <- lhsT=v,  rhs=expT
  out [t, f]   <- lhsT=attnT, rhs=W_o
Softmax runs unnormalized (no max-subtraction; scores are O(1) by
construction and exp(-1e9)=0), with the denominator computed by a ones-vector
matmul accumulated in PSUM and applied after PV via a K=1 broadcast matmul.
RoPE's rotate-half is a partition shift, done for free in the DMA that loads
q/k back from scratch, with the sign folded into the host-built sin table.
DMAs are batched into multi-dim-AP transfers (the HWDGE issue path costs
~625ns per DMA instruction, so many small DMAs throttle the PE).
"""
import numpy as np

import concourse.bass as bass  # noqa: F401  (AP types come via tile/bacc)
import concourse.tile as tile
from contextlib import ExitStack
from concourse import bacc, mybir
from concourse import bass_utils

F32 = mybir.dt.float32
F32R = mybir.dt.float32r
BF16 = mybir.dt.bfloat16
AF = mybir.ActivationFunctionType
ALU = mybir.AluOpType

B, S, H, NH = 4, 1024, 4096, 32
D = H // NH          # 128
T = B * S            # 4096 tokens
N_CORES = 8
HC = NH // N_CORES   # 4 heads per core
SCALE = float(1.0 / np.sqrt(D))
ROPE_BASE = 10000.0

TB = 256             # phase-1 token block (moving dim; >=256 keeps f32r at full rate)
NTB = T // TB        # 16
KT = H // 128        # 32 contraction tiles

_CACHE = {}


def _build_module(phases=("p1", "p2")):
    nc = bacc.Bacc("TRN2", target_bir_lowering=False, debug=False,
                   num_devices=N_CORES)

    xT = nc.dram_tensor("xT", [H, T], F32R, kind="ExternalInput").ap()
    wqk = nc.dram_tensor("wqk", [H, 2 * HC * D], F32R, kind="ExternalInput").ap()
    wv = nc.dram_tensor("wv", [H, HC * D], F32R, kind="ExternalInput").ap()
    wo = nc.dram_tensor("wo", [HC * D, H], F32R, kind="ExternalInput").ap()
    cosT = nc.dram_tensor("cosT", [D, T], F32, kind="ExternalInput").ap()
    sinS = nc.dram_tensor("sinS", [D, T], F32, kind="ExternalInput").ap()
    maskT = nc.dram_tensor("maskT", [B, S, S], BF16, kind="ExternalInput").ap()
    out_p = nc.dram_tensor("out_p", [T, H], F32, kind="ExternalOutput").ap()
    ones128 = nc.inline_tensor(np.ones((128, 1), np.float32), "ones128").ap().bitcast(F32R)
    ones1 = nc.inline_tensor(np.ones((1, 128), np.float32), "ones1").ap().bitcast(F32R)

    with tile.TileContext(nc) as tc, \
         nc.allow_low_precision(reason="tf32 matmuls; verified against reference"):
        with ExitStack() as octx:
            dram = octx.enter_context(tc.tile_pool(name="dram", bufs=1, space="DRAM"))
            cpool = octx.enter_context(tc.tile_pool(name="consts", bufs=1))
            # scratch: qkT rows (pass p, m): [q_2p, k_2p, q_2p+1, k_2p+1]
            qkT_d = dram.tile([2 * HC * D, T], F32R)
            v_d = dram.tile([T, HC * D], F32R)

            o128 = cpool.tile([128, 1], F32R)
            nc.sync.dma_start(o128[:], ones128[:])
            o1 = cpool.tile([1, 128], F32R)
            nc.sync.dma_start(o1[:], ones1[:])

            # ---------------- Phase 1: QKV projection ----------------
            if "p1" in phases:
              with ExitStack() as ctx:
                wpool = ctx.enter_context(tc.tile_pool(name="p1w", bufs=1))
                xpool = ctx.enter_context(tc.tile_pool(name="p1x", bufs=2))
                opool = ctx.enter_context(tc.tile_pool(name="p1o", bufs=2))
                pqk = ctx.enter_context(tc.tile_pool(name="p1pqk", bufs=4, space="PSUM"))
                pv = ctx.enter_context(tc.tile_pool(name="p1pv", bufs=2, space="PSUM"))

                for p in range(2):
                    # resident weights, one DMA each: sbuf [128, kk*fdim + f]
                    wqk_a = wpool.tile([128, KT * 512], F32R, tag="wqk")
                    nc.sync.dma_start(
                        wqk_a[:].rearrange("p (kk f) -> p kk f", kk=KT),
                        wqk[:, p * 512:(p + 1) * 512]
                            .rearrange("(kk p) f -> p kk f", p=128))
                    wv_a = wpool.tile([128, KT * 256], F32R, tag="wv")
                    nc.sync.dma_start(
                        wv_a[:].rearrange("p (kk f) -> p kk f", kk=KT),
                        wv[:, p * 256:(p + 1) * 256]
                            .rearrange("(kk p) f -> p kk f", p=128))

                    for tb in range(NTB):
                        t0 = tb * TB
                        xall = xpool.tile([128, KT * TB], F32R, tag="x")
                        nc.sync.dma_start(
                            xall[:].rearrange("p (kk t) -> p kk t", kk=KT),
                            xT[:, t0:t0 + TB].rearrange("(kk p) t -> p kk t", p=128))

                        qs_all = opool.tile([128, 4 * TB], F32R, tag="qs")
                        for m in range(4):
                            ps = pqk.tile([128, TB], F32, tag="qk")
                            for kk in range(KT):
                                nc.tensor.matmul(
                                    ps[:],
                                    wqk_a[:, kk * 512 + m * 128:kk * 512 + (m + 1) * 128],
                                    xall[:, kk * TB:(kk + 1) * TB],
                                    start=(kk == 0), stop=(kk == KT - 1))
                            nc.vector.tensor_copy(qs_all[:, m * TB:(m + 1) * TB], ps[:])
                        # one DMA: rows (p*4+m)*128 for m in 0..4
                        nc.sync.dma_start(
                            qkT_d[p * 512:(p + 1) * 512, t0:t0 + TB]
                                .rearrange("(m pp) t -> pp m t", pp=128),
                            qs_all[:].rearrange("pp (m t) -> pp m t", m=4))

                        vs_all = opool.tile([128, 2 * 256], F32R, tag="vs")
                        for mt in range(2):
                            ps = pv.tile([128, 256], F32, tag="v")
                            for kk in range(KT):
                                nc.tensor.matmul(
                                    ps[:],
                                    xall[:, kk * TB + mt * 128:kk * TB + (mt + 1) * 128],
                                    wv_a[:, kk * 256:(kk + 1) * 256],
                                    start=(kk == 0), stop=(kk == KT - 1))
                            nc.vector.tensor_copy(vs_all[:, mt * 256:(mt + 1) * 256], ps[:])
                        nc.sync.dma_start(
                            v_d[t0:t0 + TB, p * 256:(p + 1) * 256]
                                .rearrange("(mt pp) f -> pp mt f", pp=128),
                            vs_all[:].rearrange("pp (mt f) -> pp mt f", mt=2))

            # ---------------- Phase 2+3: attention + W_o ----------------
            if "p2" in phases:
              with ExitStack() as ctx:
                wopool = ctx.enter_context(tc.tile_pool(name="p2wo", bufs=1))
                mpool = ctx.enter_context(tc.tile_pool(name="p2m", bufs=1))
                m2pool = ctx.enter_context(tc.tile_pool(name="p2m2", bufs=2))
                tpool = ctx.enter_context(tc.tile_pool(name="p2t", bufs=2))
                rpool = ctx.enter_context(tc.tile_pool(name="p2r", bufs=1))
                tmpool = ctx.enter_context(tc.tile_pool(name="p2tm", bufs=1))
                epool = ctx.enter_context(tc.tile_pool(name="p2e", bufs=3))
                efpool = ctx.enter_context(tc.tile_pool(name="p2ef", bufs=4))
                apool = ctx.enter_context(tc.tile_pool(name="p2a", bufs=1))
                opool = ctx.enter_context(tc.tile_pool(name="p2o", bufs=2))
                ps_s = ctx.enter_context(tc.tile_pool(name="p2ps", bufs=2, space="PSUM"))
                ps_bc = ctx.enter_context(tc.tile_pool(name="p2pbc", bufs=1, space="PSUM"))
                ps_d = ctx.enter_context(tc.tile_pool(name="p2pd", bufs=1, space="PSUM"))
                ps_av = ctx.enter_context(tc.tile_pool(name="p2pav", bufs=2, space="PSUM"))
                ps_o = ctx.enter_context(tc.tile_pool(name="p2po", bufs=2, space="PSUM"))

                # W_o resident: one tile; DMA issued after the first head's
                # input loads so attention starts immediately
                wo_a = wopool.tile([128, HC * H], F32R, tag="wo")
                wo_loaded = [False]

                def load_wo():
                    nc.sync.dma_start(
                        wo_a[:].rearrange("p (l f) -> p l f", l=HC),
                        wo.rearrange("(l p) f -> p l f", p=128))
                    wo_loaded[0] = True

                for b in range(B):
                    bs = b * S
                    mask_a = m2pool.tile([128, 4 * S], BF16, tag="maskA")
                    nc.sync.dma_start(
                        mask_a[:].rearrange("p (mt t) -> p mt t", mt=4),
                        maskT[b, 0:512].rearrange("(mt p) t -> p mt t", p=128))
                    mask_bb = mpool.tile([128, 4 * S], BF16, tag="maskB")
                    nc.sync.dma_start(
                        mask_bb[:].rearrange("p (mt t) -> p mt t", mt=4),
                        maskT[b, 512:1024].rearrange("(mt p) t -> p mt t", p=128))
                    mask_halves = (mask_a, mask_bb)
                    cos_b = m2pool.tile([128, S], F32, tag="cos")
                    nc.sync.dma_start(cos_b[:], cosT[:, bs:bs + S])
                    sin_b = m2pool.tile([128, S], F32, tag="sin")
                    nc.sync.dma_start(sin_b[:], sinS[:, bs:bs + S])

                    attn_t = []
                    for l in range(HC):
                        rq = (4 * (l // 2) + 2 * (l % 2)) * 128
                        vcol = (l // 2) * 256 + (l % 2) * 128

                        # combined q,k raw load: [p, j(q/k), t] (1 DMA)
                        kq = rpool.tile([128, 2 * S], F32R, tag="kqraw")
                        nc.sync.dma_start(
                            kq[:].rearrange("p (j t) -> p j t", j=2),
                            qkT_d[rq:rq + 256, bs:bs + S]
                                .rearrange("(j p) t -> p j t", p=128))
                        # partition-rotated copy (4 DMAs: per half x per j)
                        rot = rpool.tile([128, 2 * S], F32R, tag="kqrot")
                        for j in range(2):
                            r0 = rq + j * 128
                            nc.sync.dma_start(rot[0:64, j * S:(j + 1) * S],
                                              qkT_d[r0 + 64:r0 + 128, bs:bs + S])
                            nc.sync.dma_start(rot[64:128, j * S:(j + 1) * S],
                                              qkT_d[r0:r0 + 64, bs:bs + S])
                        vt_ = tpool.tile([128, 8 * 128], F32R, tag="vt")
                        nc.sync.dma_start(
                            vt_[:].rearrange("p (kt d) -> p kt d", kt=8),
                            v_d[bs:bs + S, vcol:vcol + 128]
                               .rearrange("(kt p) d -> p kt d", p=128))

                        def rope(j):
                            m1 = tmpool.tile([128, S], F32, tag="m1")
                            nc.vector.tensor_tensor(
                                m1[:], kq[:, j * S:(j + 1) * S].bitcast(F32),
                                cos_b[:], op=ALU.mult)
                            m2 = tmpool.tile([128, S], F32, tag="m2")
                            nc.vector.tensor_tensor(
                                m2[:], rot[:, j * S:(j + 1) * S].bitcast(F32),
                                sin_b[:], op=ALU.mult)
                            out_t = tpool.tile([128, S], F32R,
                                               tag="qrope" if j == 0 else "krope")
                            nc.vector.tensor_tensor(out_t[:], m1[:], m2[:], op=ALU.add)
                            return out_t

                        q_rope = rope(0)
                        k_rope = rope(1)
                        if not wo_loaded[0]:
                            load_wo()

                        at = apool.tile([128, S], F32R, tag=f"attn{l}")
                        for nt in range(2):
                            nq = nt * 512
                            psd = ps_d.tile([1, 512], F32, tag="d")
                            psav = ps_av.tile([128, 512], F32, tag="av")
                            for mt in range(8):
                                pss = ps_s.tile([128, 512], F32, tag="s")
                                nc.tensor.matmul(
                                    pss[:], k_rope[:, mt * 128:(mt + 1) * 128],
                                    q_rope[:, nq:nq + 512], start=True, stop=True)
                                ef0 = epool.tile([128, 512], F32, tag="ef0")
                                nc.scalar.activation(ef0[:], pss[:], AF.Exp,
                                                     scale=SCALE)
                                ef = efpool.tile([128, 512], F32R, tag="ef")
                                mh = mask_halves[mt // 4]
                                msl = mh[:, (mt % 4) * S + nq:(mt % 4) * S + nq + 512]
                                eng = nc.vector if mt % 2 == 0 else nc.gpsimd
                                eng.tensor_tensor(ef[:], ef0[:], msl, op=ALU.mult)
                                nc.tensor.matmul(psd[:], o128[:], ef[:],
                                                 start=(mt == 0), stop=(mt == 7))
                                nc.tensor.matmul(
                                    psav[:], vt_[:, mt * 128:(mt + 1) * 128], ef[:],
                                    start=(mt == 0), stop=(mt == 7))
                            rd = epool.tile([1, 512], F32R, tag="rd")
                            nc.vector.reciprocal(rd[:], psd[:])
                            psbc = ps_bc.tile([128, 512], F32, tag="bc")
                            nc.tensor.matmul(psbc[:], o1[:], rd[:], start=True, stop=True)
                            bcs = epool.tile([128, 512], F32, tag="bcs")
                            nc.vector.tensor_copy(bcs[:], psbc[:])
                            nc.vector.tensor_tensor(at[:, nq:nq + 512], psav[:],
                                                    bcs[:], op=ALU.mult)
                        attn_t.append(at)

                    # W_o partial for batch b's tokens (half-row output tiles)
                    for m in range(8):
                        for half in range(4):
                            os_ = opool.tile([128, 1024], F32, tag="os")
                            for n in range(2):
                                nf = half * 1024 + n * 512
                                pso = ps_o.tile([128, 512], F32, tag="o")
                                for l in range(HC):
                                    nc.tensor.matmul(
                                        pso[:], attn_t[l][:, m * 128:(m + 1) * 128],
                                        wo_a[:, l * H + nf:l * H + nf + 512],
                                        start=(l == 0), stop=(l == HC - 1))
                                if n % 2 == 0:
                                    nc.vector.tensor_copy(
                                        os_[:, n * 512:(n + 1) * 512], pso[:])
                                else:
                                    nc.scalar.copy(
                                        os_[:, n * 512:(n + 1) * 512], pso[:])
                            nc.sync.dma_start(
                                out_p[bs + m * 128:bs + (m + 1) * 128,
                                      half * 1024:(half + 1) * 1024], os_[:])
    nc.compile()
    return nc


def _host_prep(hidden_states, W_pack, W_o, attention_mask, position_ids):
    import ml_dtypes
    hidden_states = np.asarray(hidden_states, dtype=np.float32)
    W_pack = np.asarray(W_pack, dtype=np.float32)
    W_o = np.asarray(W_o, dtype=np.float32)
    attention_mask = np.asarray(attention_mask, dtype=np.float32)
    pos = np.asarray(position_ids)

    xT = np.ascontiguousarray(hidden_states.reshape(T, H).T)
    # exp(mask): softmax mask applied multiplicatively after exp
    maskT = np.ascontiguousarray(
        np.exp(attention_mask[:, 0].transpose(0, 2, 1)).astype(ml_dtypes.bfloat16))

    inv = (1.0 / (ROPE_BASE ** (np.arange(0, D, 2, dtype=np.float64) / D)))
    inv = np.concatenate([inv, inv])                       # [D]
    ang = pos.astype(np.float64).reshape(T)[None, :] * inv[:, None]   # [D, T]
    cosT = np.cos(ang).astype(np.float32)
    sinT = np.sin(ang).astype(np.float32)
    sinS = sinT.copy()
    sinS[:64] = -sinT[:64]
    cosT = np.ascontiguousarray(cosT)
    sinS = np.ascontiguousarray(sinS)

    in_maps = []
    for c in range(N_CORES):
        h0 = c * HC
        # wqk column order per pass p: [q_{2p}, k_{2p}, q_{2p+1}, k_{2p+1}]
        qcols = [W_pack[:, (h0 + l) * D:(h0 + l + 1) * D] for l in range(HC)]
        kcols = [W_pack[:, H + (h0 + l) * D:H + (h0 + l + 1) * D] for l in range(HC)]
        vcols = [W_pack[:, 2 * H + (h0 + l) * D:2 * H + (h0 + l + 1) * D]
                 for l in range(HC)]
        wqk_np = np.ascontiguousarray(np.concatenate(
            [qcols[0], kcols[0], qcols[1], kcols[1],
             qcols[2], kcols[2], qcols[3], kcols[3]], axis=1))
        wv_np = np.ascontiguousarray(np.concatenate(vcols, axis=1))
        wo_np = np.ascontiguousarray(W_o[h0 * D:(h0 + HC) * D, :])
        in_maps.append({
            "xT": xT, "wqk": wqk_np, "wv": wv_np, "wo": wo_np,
            "cosT": cosT, "sinS": sinS, "maskT": maskT,
        })
    return in_maps


def kernel(hidden_states, W_pack, W_o, attention_mask, position_ids):
    if "nc" not in _CACHE:
        _CACHE["nc"] = _build_module()
    nc = _CACHE["nc"]
    in_maps = _host_prep(hidden_states, W_pack, W_o, attention_mask, position_ids)
    res = bass_utils.run_bass_kernel_spmd(nc, in_maps, core_ids=list(range(N_CORES)))
    out = res.results[0]["out_p"].astype(np.float64)
    for c in range(1, N_CORES):
        out += res.results[c]["out_p"]
    return out.reshape(B, S, H).astype(np.float32)


# revision 10
# speedup vs baseline: 1.1539x; 1.0131x over previous
"""Trainium2 Bass kernel for nn_Attention_60567628808865.

Dense transformer attention block (B=4, S=1024, H=4096, NH=32, D=128):
  qkv = x @ W_pack; RoPE(q, k); causal-masked softmax attention; out @ W_o.

Sharding: tensor-parallel over heads across 8 NeuronCores. Each core computes
4 heads end-to-end (QKV projection with its W_pack column slice, attention,
and its W_o row-slice partial of the output projection); the host sums the 8
partial outputs.

All matmuls run in float32r (TF32) at full PE rate; accumulation is fp32 in
PSUM. Everything on-chip works in a transposed layout (features on the
partition axis) so no transposes are needed anywhere:
  qT/kT [d, t] <- lhsT=W_qk, rhs=xT      scoresT [tk, tq] <- lhsT=kT, rhs=qT
  v [t, d]     <- lhsT=xT,   rhs=W_v     attnT [d, tq]    <- lhsT=v,  rhs=expT
  out [t, f]   <- lhsT=attnT, rhs=W_o
Softmax runs unnormalized (no max-subtraction; scores are O(1) by
construction and exp(-1e9)=0), with the denominator computed by a ones-vector
matmul accumulated in PSUM and applied after PV via a K=1 broadcast matmul.
RoPE's rotate-half is a partition shift, done for free in the DMA that loads
q/k back from scratch, with the sign folded into the host-built sin table.
DMAs are batched into multi-dim-AP transfers (the HWDGE issue path costs
~625ns per DMA instruction, so many small DMAs throttle the PE).
"""
import numpy as np

import concourse.bass as bass  # noqa: F401  (AP types come via tile/bacc)
import concourse.tile as tile
from contextlib import ExitStack
from concourse import bacc, mybir
from concourse import bass_utils

F32 = mybir.dt.float32
F32R = mybir.dt.float32r
BF16 = mybir.dt.bfloat16
AF = mybir.ActivationFunctionType
ALU = mybir.AluOpType

B, S, H, NH = 4, 1024, 4096, 32
D = H // NH          # 128
T = B * S            # 4096 tokens
N_CORES = 8
HC = NH // N_CORES   # 4 heads per core
SCALE = float(1.0 / np.sqrt(D))
ROPE_BASE = 10000.0

TB = 256             # phase-1 token block (moving dim; >=256 keeps f32r at full rate)
NTB = T // TB        # 16
KT = H // 128        # 32 contraction tiles

_CACHE = {}


def _build_module(phases=("p1", "p2")):
    nc = bacc.Bacc("TRN2", target_bir_lowering=False, debug=False,
                   num_devices=N_CORES)

    xT = nc.dram_tensor("xT", [H, T], F32R, kind="ExternalInput").ap()
    wqk = nc.dram_tensor("wqk", [H, 2 * HC * D], F32R, kind="ExternalInput").ap()
    wv = nc.dram_tensor("wv", [H, HC * D], F32R, kind="ExternalInput").ap()
    wo = nc.dram_tensor("wo", [HC * D, H], F32R, kind="ExternalInput").ap()
    cosT = nc.dram_tensor("cosT", [D, T], F32, kind="ExternalInput").ap()
    sinS = nc.dram_tensor("sinS", [D, T], F32, kind="ExternalInput").ap()
    maskT = nc.dram_tensor("maskT", [B, S, S], BF16, kind="ExternalInput").ap()
    out_p = nc.dram_tensor("out_p", [T, H], F32, kind="ExternalOutput").ap()
    ones128 = nc.inline_tensor(np.ones((128, 1), np.float32), "ones128").ap().bitcast(F32R)
    ones1 = nc.inline_tensor(np.ones((1, 128), np.float32), "ones1").ap().bitcast(F32R)

    with tile.TileContext(nc) as tc, \
         nc.allow_low_precision(reason="tf32 matmuls; verified against reference"):
        with ExitStack() as octx:
            dram = octx.enter_context(tc.tile_pool(name="dram", bufs=1, space="DRAM"))
            cpool = octx.enter_context(tc.tile_pool(name="consts", bufs=1))
            # scratch: qkT rows (pass p, m): [q_2p, k_2p, q_2p+1, k_2p+1]
            qkT_d = dram.tile([2 * HC * D, T], F32R)
            v_d = dram.tile([T, HC * D], F32R)

            o128 = cpool.tile([128, 1], F32R)
            nc.sync.dma_start(o128[:], ones128[:])
            o1 = cpool.tile([1, 128], F32R)
            nc.sync.dma_start(o1[:], ones1[:])

            # ---------------- Phase 1: QKV projection ----------------
            if "p1" in phases:
              with ExitStack() as ctx:
                wpool = ctx.enter_context(tc.tile_pool(name="p1w", bufs=1))
                xpool = ctx.enter_context(tc.tile_pool(name="p1x", bufs=2))
                opool = ctx.enter_context(tc.tile_pool(name="p1o", bufs=2))
                pqk = ctx.enter_context(tc.tile_pool(name="p1pqk", bufs=6, space="PSUM"))
                pv = ctx.enter_context(tc.tile_pool(name="p1pv", bufs=2, space="PSUM"))

                for p in range(2):
                    # resident weights, split into half-K DMAs so the first
                    # accumulation chains can start at half-load
                    KH = KT // 2
                    wqk_a = wpool.tile([128, KT * 512], F32R, tag="wqk")
                    wv_a = wpool.tile([128, KT * 256], F32R, tag="wv")
                    for kh in range(2):
                        nc.sync.dma_start(
                            wqk_a[:, kh * KH * 512:(kh + 1) * KH * 512]
                                .rearrange("p (kk f) -> p kk f", kk=KH),
                            wqk[kh * KH * 128:(kh + 1) * KH * 128,
                                p * 512:(p + 1) * 512]
                                .rearrange("(kk p) f -> p kk f", p=128))
                        nc.sync.dma_start(
                            wv_a[:, kh * KH * 256:(kh + 1) * KH * 256]
                                .rearrange("p (kk f) -> p kk f", kk=KH),
                            wv[kh * KH * 128:(kh + 1) * KH * 128,
                               p * 256:(p + 1) * 256]
                                .rearrange("(kk p) f -> p kk f", p=128))

                    for tb in range(NTB):
                        t0 = tb * TB
                        xall = xpool.tile([128, KT * TB], F32R, tag="x")
                        for kh in range(2):
                            nc.sync.dma_start(
                                xall[:, kh * 16 * TB:(kh + 1) * 16 * TB]
                                    .rearrange("p (kk t) -> p kk t", kk=16),
                                xT[kh * 2048:(kh + 1) * 2048, t0:t0 + TB]
                                    .rearrange("(kk p) t -> p kk t", p=128))

                        qs_all = opool.tile([128, 4 * TB], F32R, tag="qs")
                        for m in range(4):
                            ps = pqk.tile([128, TB], F32, tag="qk")
                            for kk in range(KT):
                                nc.tensor.matmul(
                                    ps[:],
                                    wqk_a[:, kk * 512 + m * 128:kk * 512 + (m + 1) * 128],
                                    xall[:, kk * TB:(kk + 1) * TB],
                                    start=(kk == 0), stop=(kk == KT - 1))
                            nc.vector.tensor_copy(qs_all[:, m * TB:(m + 1) * TB], ps[:])
                        # one DMA: rows (p*4+m)*128 for m in 0..4
                        nc.sync.dma_start(
                            qkT_d[p * 512:(p + 1) * 512, t0:t0 + TB]
                                .rearrange("(m pp) t -> pp m t", pp=128),
                            qs_all[:].rearrange("pp (m t) -> pp m t", m=4))

                        vs_all = opool.tile([128, 2 * 256], F32R, tag="vs")
                        for mt in range(2):
                            ps = pv.tile([128, 256], F32, tag="v")
                            for kk in range(KT):
                                nc.tensor.matmul(
                                    ps[:],
                                    xall[:, kk * TB + mt * 128:kk * TB + (mt + 1) * 128],
                                    wv_a[:, kk * 256:(kk + 1) * 256],
                                    start=(kk == 0), stop=(kk == KT - 1))
                            nc.vector.tensor_copy(vs_all[:, mt * 256:(mt + 1) * 256], ps[:])
                        nc.sync.dma_start(
                            v_d[t0:t0 + TB, p * 256:(p + 1) * 256]
                                .rearrange("(mt pp) f -> pp mt f", pp=128),
                            vs_all[:].rearrange("pp (mt f) -> pp mt f", mt=2))

            # ---------------- Phase 2+3: attention + W_o ----------------
            if "p2" in phases:
              with ExitStack() as ctx:
                wopool = ctx.enter_context(tc.tile_pool(name="p2wo", bufs=1))
                mpool = ctx.enter_context(tc.tile_pool(name="p2m", bufs=1))
                m2pool = ctx.enter_context(tc.tile_pool(name="p2m2", bufs=2))
                tpool = ctx.enter_context(tc.tile_pool(name="p2t", bufs=2))
                rpool = ctx.enter_context(tc.tile_pool(name="p2r", bufs=1))
                tmpool = ctx.enter_context(tc.tile_pool(name="p2tm", bufs=1))
                epool = ctx.enter_context(tc.tile_pool(name="p2e", bufs=3))
                efpool = ctx.enter_context(tc.tile_pool(name="p2ef", bufs=4))
                apool = ctx.enter_context(tc.tile_pool(name="p2a", bufs=1))
                opool = ctx.enter_context(tc.tile_pool(name="p2o", bufs=2))
                ps_s = ctx.enter_context(tc.tile_pool(name="p2ps", bufs=2, space="PSUM"))
                ps_bc = ctx.enter_context(tc.tile_pool(name="p2pbc", bufs=1, space="PSUM"))
                ps_d = ctx.enter_context(tc.tile_pool(name="p2pd", bufs=1, space="PSUM"))
                ps_av = ctx.enter_context(tc.tile_pool(name="p2pav", bufs=2, space="PSUM"))
                ps_o = ctx.enter_context(tc.tile_pool(name="p2po", bufs=2, space="PSUM"))

                # W_o resident: one tile; DMA issued after the first head's
                # input loads so attention starts immediately
                wo_a = wopool.tile([128, HC * H], F32R, tag="wo")
                wo_loaded = [False]

                def load_wo():
                    nc.sync.dma_start(
                        wo_a[:].rearrange("p (l f) -> p l f", l=HC),
                        wo.rearrange("(l p) f -> p l f", p=128))
                    wo_loaded[0] = True

                for b in range(B):
                    bs = b * S
                    mask_a = m2pool.tile([128, 4 * S], BF16, tag="maskA")
                    nc.sync.dma_start(
                        mask_a[:].rearrange("p (mt t) -> p mt t", mt=4),
                        maskT[b, 0:512].rearrange("(mt p) t -> p mt t", p=128))
                    mask_bb = mpool.tile([128, 4 * S], BF16, tag="maskB")
                    nc.sync.dma_start(
                        mask_bb[:].rearrange("p (mt t) -> p mt t", mt=4),
                        maskT[b, 512:1024].rearrange("(mt p) t -> p mt t", p=128))
                    mask_halves = (mask_a, mask_bb)
                    cos_b = m2pool.tile([128, S], F32, tag="cos")
                    nc.sync.dma_start(cos_b[:], cosT[:, bs:bs + S])
                    sin_b = m2pool.tile([128, S], F32, tag="sin")
                    nc.sync.dma_start(sin_b[:], sinS[:, bs:bs + S])

                    attn_t = []
                    for l in range(HC):
                        rq = (4 * (l // 2) + 2 * (l % 2)) * 128
                        vcol = (l // 2) * 256 + (l % 2) * 128

                        # combined q,k raw load: [p, j(q/k), t] (1 DMA)
                        kq = rpool.tile([128, 2 * S], F32R, tag="kqraw")
                        nc.sync.dma_start(
                            kq[:].rearrange("p (j t) -> p j t", j=2),
                            qkT_d[rq:rq + 256, bs:bs + S]
                                .rearrange("(j p) t -> p j t", p=128))
                        # partition-rotated copy (4 DMAs: per half x per j)
                        rot = rpool.tile([128, 2 * S], F32R, tag="kqrot")
                        for j in range(2):
                            r0 = rq + j * 128
                            nc.sync.dma_start(rot[0:64, j * S:(j + 1) * S],
                                              qkT_d[r0 + 64:r0 + 128, bs:bs + S])
                            nc.sync.dma_start(rot[64:128, j * S:(j + 1) * S],
                                              qkT_d[r0:r0 + 64, bs:bs + S])
                        vt_ = tpool.tile([128, 8 * 128], F32R, tag="vt")
                        nc.sync.dma_start(
                            vt_[:].rearrange("p (kt d) -> p kt d", kt=8),
                            v_d[bs:bs + S, vcol:vcol + 128]
                               .rearrange("(kt p) d -> p kt d", p=128))

                        def rope(j):
                            m1 = tmpool.tile([128, S], F32, tag="m1")
                            nc.vector.tensor_tensor(
                                m1[:], kq[:, j * S:(j + 1) * S].bitcast(F32),
                                cos_b[:], op=ALU.mult)
                            m2 = tmpool.tile([128, S], F32, tag="m2")
                            nc.vector.tensor_tensor(
                                m2[:], rot[:, j * S:(j + 1) * S].bitcast(F32),
                                sin_b[:], op=ALU.mult)
                            out_t = tpool.tile([128, S], F32R,
                                               tag="qrope" if j == 0 else "krope")
                            nc.vector.tensor_tensor(out_t[:], m1[:], m2[:], op=ALU.add)
                            return out_t

                        q_rope = rope(0)
                        k_rope = rope(1)
                        if not wo_loaded[0]:
                            load_wo()

                        at = apool.tile([128, S], F32R, tag=f"attn{l}")
                        for nt in range(2):
                            nq = nt * 512
                            psd = ps_d.tile([1, 512], F32, tag="d")
                            psav = ps_av.tile([128, 512], F32, tag="av")
                            for mt in range(8):
                                pss = ps_s.tile([128, 512], F32, tag="s")
                                nc.tensor.matmul(
                                    pss[:], k_rope[:, mt * 128:(mt + 1) * 128],
                                    q_rope[:, nq:nq + 512], start=True, stop=True)
                                ef0 = epool.tile([128, 512], F32, tag="ef0")
                                nc.scalar.activation(ef0[:], pss[:], AF.Exp,
                                                     scale=SCALE)
                                ef = efpool.tile([128, 512], F32R, tag="ef")
                                mh = mask_halves[mt // 4]
                                msl = mh[:, (mt % 4) * S + nq:(mt % 4) * S + nq + 512]
                                eng = nc.vector if mt % 2 == 0 else nc.gpsimd
                                eng.tensor_tensor(ef[:], ef0[:], msl, op=ALU.mult)
                                nc.tensor.matmul(psd[:], o128[:], ef[:],
                                                 start=(mt == 0), stop=(mt == 7))
                                nc.tensor.matmul(
                                    psav[:], vt_[:, mt * 128:(mt + 1) * 128], ef[:],
                                    start=(mt == 0), stop=(mt == 7))
                            rd = epool.tile([1, 512], F32R, tag="rd")
                            nc.vector.reciprocal(rd[:], psd[:])
                            psbc = ps_bc.tile([128, 512], F32, tag="bc")
                            nc.tensor.matmul(psbc[:], o1[:], rd[:], start=True, stop=True)
                            bcs = epool.tile([128, 512], F32, tag="bcs")
                            nc.vector.tensor_copy(bcs[:], psbc[:])
                            nc.vector.tensor_tensor(at[:, nq:nq + 512], psav[:],
                                                    bcs[:], op=ALU.mult)
                        attn_t.append(at)

                    # W_o partial for batch b's tokens (half-row output tiles)
                    for m in range(8):
                        for half in range(4):
                            os_ = opool.tile([128, 1024], F32, tag="os")
                            for n in range(2):
                                nf = half * 1024 + n * 512
                                pso = ps_o.tile([128, 512], F32, tag="o")
                                for l in range(HC):
                                    nc.tensor.matmul(
                                        pso[:], attn_t[l][:, m * 128:(m + 1) * 128],
                                        wo_a[:, l * H + nf:l * H + nf + 512],
                                        start=(l == 0), stop=(l == HC - 1))
                                if n % 2 == 0:
                                    nc.vector.tensor_copy(
                                        os_[:, n * 512:(n + 1) * 512], pso[:])
                                else:
                                    nc.scalar.copy(
                                        os_[:, n * 512:(n + 1) * 512], pso[:])
                            nc.sync.dma_start(
                                out_p[bs + m * 128:bs + (m + 1) * 128,
                                      half * 1024:(half + 1) * 1024], os_[:])
    nc.compile()
    return nc


def _host_prep(hidden_states, W_pack, W_o, attention_mask, position_ids):
    import ml_dtypes
    hidden_states = np.asarray(hidden_states, dtype=np.float32)
    W_pack = np.asarray(W_pack, dtype=np.float32)
    W_o = np.asarray(W_o, dtype=np.float32)
    attention_mask = np.asarray(attention_mask, dtype=np.float32)
    pos = np.asarray(position_ids)

    xT = np.ascontiguousarray(hidden_states.reshape(T, H).T)
    # exp(mask): softmax mask applied multiplicatively after exp
    maskT = np.ascontiguousarray(
        np.exp(attention_mask[:, 0].transpose(0, 2, 1)).astype(ml_dtypes.bfloat16))

    inv = (1.0 / (ROPE_BASE ** (np.arange(0, D, 2, dtype=np.float64) / D)))
    inv = np.concatenate([inv, inv])                       # [D]
    ang = pos.astype(np.float64).reshape(T)[None, :] * inv[:, None]   # [D, T]
    cosT = np.cos(ang).astype(np.float32)
    sinT = np.sin(ang).astype(np.float32)
    sinS = sinT.copy()
    sinS[:64] = -sinT[:64]
    cosT = np.ascontiguousarray(cosT)
    sinS = np.ascontiguousarray(sinS)

    in_maps = []
    for c in range(N_CORES):
        h0 = c * HC
        # wqk column order per pass p: [q_{2p}, k_{2p}, q_{2p+1}, k_{2p+1}]
        qcols = [W_pack[:, (h0 + l) * D:(h0 + l + 1) * D] for l in range(HC)]
        kcols = [W_pack[:, H + (h0 + l) * D:H + (h0 + l + 1) * D] for l in range(HC)]
        vcols = [W_pack[:, 2 * H + (h0 + l) * D:2 * H + (h0 + l + 1) * D]
                 for l in range(HC)]
        wqk_np = np.ascontiguousarray(np.concatenate(
            [qcols[0], kcols[0], qcols[1], kcols[1],
             qcols[2], kcols[2], qcols[3], kcols[3]], axis=1))
        wv_np = np.ascontiguousarray(np.concatenate(vcols, axis=1))
        wo_np = np.ascontiguousarray(W_o[h0 * D:(h0 + HC) * D, :])
        in_maps.append({
            "xT": xT, "wqk": wqk_np, "wv": wv_np, "wo": wo_np,
            "cosT": cosT, "sinS": sinS, "maskT": maskT,
        })
    return in_maps


def kernel(hidden_states, W_pack, W_o, attention_mask, position_ids):
    if "nc" not in _CACHE:
        _CACHE["nc"] = _build_module()
    nc = _CACHE["nc"]
    in_maps = _host_prep(hidden_states, W_pack, W_o, attention_mask, position_ids)
    res = bass_utils.run_bass_kernel_spmd(nc, in_maps, core_ids=list(range(N_CORES)))
    out = res.results[0]["out_p"].astype(np.float64)
    for c in range(1, N_CORES):
        out += res.results[c]["out_p"]
    return out.reshape(B, S, H).astype(np.float32)


# revision 22
# speedup vs baseline: 1.2432x; 1.0774x over previous
"""Trainium2 Bass kernel for nn_Attention_60567628808865.

Dense transformer attention block (B=4, S=1024, H=4096, NH=32, D=128):
  qkv = x @ W_pack; RoPE(q, k); causal-masked softmax attention; out @ W_o.

Sharding: tensor-parallel over heads across 8 NeuronCores. Each core computes
4 heads end-to-end (QKV projection with its W_pack column slice, attention,
and its W_o row-slice partial of the output projection); the host sums the 8
partial outputs.

All matmuls run in float32r (TF32) at full PE rate; accumulation is fp32 in
PSUM. Everything on-chip works in a transposed layout (features on the
partition axis) so no transposes are needed anywhere:
  qT/kT [d, t] <- lhsT=W_qk, rhs=xT      scoresT [tk, tq] <- lhsT=kT, rhs=qT
  v [t, d]     <- lhsT=xT,   rhs=W_v     attnT [d, tq]    <- lhsT=v,  rhs=expT
  out [t, f]   <- lhsT=attnT, rhs=W_o
Softmax runs unnormalized (no max-subtraction; scores are O(1) by
construction and exp(-1e9)=0), with the denominator computed by a ones-vector
matmul accumulated in PSUM and applied after PV via a K=1 broadcast matmul.
RoPE's rotate-half is a partition shift, done for free in the DMA that loads
q/k back from scratch, with the sign folded into the host-built sin table.
DMAs are batched into multi-dim-AP transfers (the HWDGE issue path costs
~625ns per DMA instruction, so many small DMAs throttle the PE).
"""
import numpy as np

import concourse.bass as bass  # noqa: F401  (AP types come via tile/bacc)
import concourse.tile as tile
from contextlib import ExitStack
from concourse import bacc, mybir
from concourse import bass_utils

F32 = mybir.dt.float32
F32R = mybir.dt.float32r
BF16 = mybir.dt.bfloat16
AF = mybir.ActivationFunctionType
ALU = mybir.AluOpType

B, S, H, NH = 4, 1024, 4096, 32
D = H // NH          # 128
T = B * S            # 4096 tokens
N_CORES = 8
HC = NH // N_CORES   # 4 heads per core
SCALE = float(1.0 / np.sqrt(D))
ROPE_BASE = 10000.0

TB = 256             # phase-1 token block (moving dim; >=256 keeps f32r at full rate)
NTB = T // TB        # 16
KT = H // 128        # 32 contraction tiles

_CACHE = {}


def _build_module(phases=("p1", "p2")):
    nc = bacc.Bacc("TRN2", target_bir_lowering=False, debug=False,
                   num_devices=N_CORES)

    xT = nc.dram_tensor("xT", [H, T], F32R, kind="ExternalInput").ap()
    wqk = nc.dram_tensor("wqk", [H, 2 * HC * D], F32R, kind="ExternalInput").ap()
    wv = nc.dram_tensor("wv", [H, HC * D], F32R, kind="ExternalInput").ap()
    wo = nc.dram_tensor("wo", [HC * D, H], F32R, kind="ExternalInput").ap()
    cosT = nc.dram_tensor("cosT", [D, T], F32, kind="ExternalInput").ap()
    sinS = nc.dram_tensor("sinS", [D, T], F32, kind="ExternalInput").ap()
    maskT = nc.dram_tensor("maskT", [B, S, S], BF16, kind="ExternalInput").ap()
    out_p = nc.dram_tensor("out_p", [T, H], F32, kind="ExternalOutput").ap()
    ones128 = nc.inline_tensor(np.ones((128, 1), np.float32), "ones128").ap().bitcast(F32R)
    ones1 = nc.inline_tensor(np.ones((1, 128), np.float32), "ones1").ap().bitcast(F32R)

    with tile.TileContext(nc) as tc, \
         nc.allow_low_precision(reason="tf32 matmuls; verified against reference"):
        with ExitStack() as octx:
            dram = octx.enter_context(tc.tile_pool(name="dram", bufs=1, space="DRAM"))
            cpool = octx.enter_context(tc.tile_pool(name="consts", bufs=1))
            # scratch: qkT rows (pass p, m): [q_2p, k_2p, q_2p+1, k_2p+1]
            qkT_d = dram.tile([2 * HC * D, T], F32R)
            v_d = dram.tile([T, HC * D], F32R)

            o128 = cpool.tile([128, 1], F32R)
            nc.sync.dma_start(o128[:], ones128[:])
            o1 = cpool.tile([1, 128], F32R)
            nc.sync.dma_start(o1[:], ones1[:])

            # ---------------- Phase 1: QKV projection ----------------
            if "p1" in phases:
              with ExitStack() as ctx:
                wpool = ctx.enter_context(tc.tile_pool(name="p1w", bufs=1))
                xpool = ctx.enter_context(tc.tile_pool(name="p1x", bufs=2))
                opool = ctx.enter_context(tc.tile_pool(name="p1o", bufs=2))
                cpool1 = ctx.enter_context(tc.tile_pool(name="p1cs", bufs=2))
                rpool1 = ctx.enter_context(tc.tile_pool(name="p1rope", bufs=2))
                pqk = ctx.enter_context(tc.tile_pool(name="p1pqk", bufs=6, space="PSUM"))
                pv = ctx.enter_context(tc.tile_pool(name="p1pv", bufs=2, space="PSUM"))

                for p in range(2):
                    # resident weights, split into half-K DMAs so the first
                    # accumulation chains can start at half-load
                    KH = KT // 2
                    wqk_a = wpool.tile([128, KT * 512], F32R, tag="wqk")
                    wv_a = wpool.tile([128, KT * 256], F32R, tag="wv")
                    for kh in range(2):
                        nc.sync.dma_start(
                            wqk_a[:, kh * KH * 512:(kh + 1) * KH * 512]
                                .rearrange("p (kk f) -> p kk f", kk=KH),
                            wqk[kh * KH * 128:(kh + 1) * KH * 128,
                                p * 512:(p + 1) * 512]
                                .rearrange("(kk p) f -> p kk f", p=128))
                        nc.sync.dma_start(
                            wv_a[:, kh * KH * 256:(kh + 1) * KH * 256]
                                .rearrange("p (kk f) -> p kk f", kk=KH),
                            wv[kh * KH * 128:(kh + 1) * KH * 128,
                               p * 256:(p + 1) * 256]
                                .rearrange("(kk p) f -> p kk f", p=128))

                    for tb in range(NTB):
                        t0 = tb * TB
                        cos_tb = cpool1.tile([128, TB], F32, tag="cos")
                        nc.sync.dma_start(cos_tb[:], cosT[:, t0:t0 + TB])
                        sin_tb = cpool1.tile([128, TB], F32, tag="sin")
                        nc.sync.dma_start(sin_tb[:], sinS[:, t0:t0 + TB])
                        xall = xpool.tile([128, KT * TB], F32R, tag="x")
                        for kh in range(2):
                            nc.sync.dma_start(
                                xall[:, kh * 16 * TB:(kh + 1) * 16 * TB]
                                    .rearrange("p (kk t) -> p kk t", kk=16),
                                xT[kh * 2048:(kh + 1) * 2048, t0:t0 + TB]
                                    .rearrange("(kk p) t -> p kk t", p=128))

                        qs_all = opool.tile([128, 4 * TB], F32R, tag="qs")
                        for m in range(4):
                            ps = pqk.tile([128, TB], F32, tag="qk")
                            for kk in range(KT):
                                nc.tensor.matmul(
                                    ps[:],
                                    wqk_a[:, kk * 512 + m * 128:kk * 512 + (m + 1) * 128],
                                    xall[:, kk * TB:(kk + 1) * TB],
                                    start=(kk == 0), stop=(kk == KT - 1))
                            # RoPE fused into the epilogue: rotate-half via
                            # partition-shifted copies, sign folded into sinS
                            rot = rpool1.tile([128, TB], F32, tag="rot")
                            nc.vector.tensor_copy(rot[0:64, :], ps[64:128, :])
                            nc.vector.tensor_copy(rot[64:128, :], ps[0:64, :])
                            m1_ = rpool1.tile([128, TB], F32, tag="m1")
                            nc.vector.tensor_tensor(m1_[:], ps[:], cos_tb[:],
                                                    op=ALU.mult)
                            m2_ = rpool1.tile([128, TB], F32, tag="m2")
                            nc.vector.tensor_tensor(m2_[:], rot[:], sin_tb[:],
                                                    op=ALU.mult)
                            nc.vector.tensor_tensor(qs_all[:, m * TB:(m + 1) * TB],
                                                    m1_[:], m2_[:], op=ALU.add)
                        # one DMA: rows (p*4+m)*128 for m in 0..4
                        nc.sync.dma_start(
                            qkT_d[p * 512:(p + 1) * 512, t0:t0 + TB]
                                .rearrange("(m pp) t -> pp m t", pp=128),
                            qs_all[:].rearrange("pp (m t) -> pp m t", m=4))

                        vs_all = opool.tile([128, 2 * 256], F32R, tag="vs")
                        for mt in range(2):
                            ps = pv.tile([128, 256], F32, tag="v")
                            for kk in range(KT):
                                nc.tensor.matmul(
                                    ps[:],
                                    xall[:, kk * TB + mt * 128:kk * TB + (mt + 1) * 128],
                                    wv_a[:, kk * 256:(kk + 1) * 256],
                                    start=(kk == 0), stop=(kk == KT - 1))
                            nc.vector.tensor_copy(vs_all[:, mt * 256:(mt + 1) * 256], ps[:])
                        nc.sync.dma_start(
                            v_d[t0:t0 + TB, p * 256:(p + 1) * 256]
                                .rearrange("(mt pp) f -> pp mt f", pp=128),
                            vs_all[:].rearrange("pp (mt f) -> pp mt f", mt=2))

            # ---------------- Phase 2+3: attention + W_o ----------------
            if "p2" in phases:
              with ExitStack() as ctx:
                wopool = ctx.enter_context(tc.tile_pool(name="p2wo", bufs=1))
                mpool = ctx.enter_context(tc.tile_pool(name="p2m", bufs=1))
                m2pool = ctx.enter_context(tc.tile_pool(name="p2m2", bufs=2))
                tpool = ctx.enter_context(tc.tile_pool(name="p2t", bufs=2))
                epool = ctx.enter_context(tc.tile_pool(name="p2e", bufs=3))
                efpool = ctx.enter_context(tc.tile_pool(name="p2ef", bufs=4))
                apool = ctx.enter_context(tc.tile_pool(name="p2a", bufs=2))
                opool = ctx.enter_context(tc.tile_pool(name="p2o", bufs=2))
                ps_s = ctx.enter_context(tc.tile_pool(name="p2ps", bufs=2, space="PSUM"))
                ps_bc = ctx.enter_context(tc.tile_pool(name="p2pbc", bufs=1, space="PSUM"))
                ps_d = ctx.enter_context(tc.tile_pool(name="p2pd", bufs=1, space="PSUM"))
                ps_av = ctx.enter_context(tc.tile_pool(name="p2pav", bufs=2, space="PSUM"))
                ps_o = ctx.enter_context(tc.tile_pool(name="p2po", bufs=2, space="PSUM"))

                # W_o resident: one tile; DMA issued after the first head's
                # input loads so attention starts immediately
                wo_a = wopool.tile([128, HC * H], F32R, tag="wo")
                wo_loaded = [False]

                def load_wo():
                    if not wo_loaded[0]:
                        nc.sync.dma_start(
                            wo_a[:].rearrange("p (l f) -> p l f", l=HC),
                            wo.rearrange("(l p) f -> p l f", p=128))
                        wo_loaded[0] = True

                for b in range(B):
                    bs = b * S
                    mask_a = m2pool.tile([128, 4 * S], BF16, tag="maskA")
                    nc.sync.dma_start(
                        mask_a[:].rearrange("p (mt t) -> p mt t", mt=4),
                        maskT[b, 0:512].rearrange("(mt p) t -> p mt t", p=128))
                    mask_bb = m2pool.tile([128, 4 * S], BF16, tag="maskB")
                    nc.sync.dma_start(
                        mask_bb[:].rearrange("p (mt t) -> p mt t", mt=4),
                        maskT[b, 512:1024].rearrange("(mt p) t -> p mt t", p=128))
                    mask_halves = (mask_a, mask_bb)

                    attn_t = []
                    for l in range(HC):
                        rq = (4 * (l // 2) + 2 * (l % 2)) * 128
                        vcol = (l // 2) * 256 + (l % 2) * 128

                        # rope'd q,k load: [p, j(q/k), t] (1 DMA)
                        kq = tpool.tile([128, 2 * S], F32R, tag="kqraw")
                        nc.sync.dma_start(
                            kq[:].rearrange("p (j t) -> p j t", j=2),
                            qkT_d[rq:rq + 256, bs:bs + S]
                                .rearrange("(j p) t -> p j t", p=128))
                        vt_ = tpool.tile([128, 8 * 128], F32R, tag="vt")
                        nc.sync.dma_start(
                            vt_[:].rearrange("p (kt d) -> p kt d", kt=8),
                            v_d[bs:bs + S, vcol:vcol + 128]
                               .rearrange("(kt p) d -> p kt d", p=128))
                        q_rope = kq[:, 0:S]
                        k_rope = kq[:, S:2 * S]
                        load_wo()

                        at = apool.tile([128, S], F32R, tag=f"attn{l}")
                        for nt in range(2):
                            nq = nt * 512
                            psd = ps_d.tile([1, 512], F32, tag="d")
                            psav = ps_av.tile([128, 512], F32, tag="av")
                            for mt in range(8):
                                pss = ps_s.tile([128, 512], F32, tag="s")
                                nc.tensor.matmul(
                                    pss[:], k_rope[:, mt * 128:(mt + 1) * 128],
                                    q_rope[:, nq:nq + 512], start=True, stop=True)
                                ef0 = epool.tile([128, 512], F32, tag="ef0")
                                nc.scalar.activation(ef0[:], pss[:], AF.Exp,
                                                     scale=SCALE)
                                ef = efpool.tile([128, 512], F32R, tag="ef")
                                mh = mask_halves[mt // 4]
                                msl = mh[:, (mt % 4) * S + nq:(mt % 4) * S + nq + 512]
                                eng = nc.gpsimd if mt % 4 == 3 else nc.vector
                                eng.tensor_tensor(ef[:], ef0[:], msl, op=ALU.mult)
                                nc.tensor.matmul(psd[:], o128[:], ef[:],
                                                 start=(mt == 0), stop=(mt == 7))
                                nc.tensor.matmul(
                                    psav[:], vt_[:, mt * 128:(mt + 1) * 128], ef[:],
                                    start=(mt == 0), stop=(mt == 7))
                            rd = epool.tile([1, 512], F32R, tag="rd")
                            nc.vector.reciprocal(rd[:], psd[:])
                            psbc = ps_bc.tile([128, 512], F32, tag="bc")
                            nc.tensor.matmul(psbc[:], o1[:], rd[:], start=True, stop=True)
                            bcs = epool.tile([128, 512], F32, tag="bcs")
                            nc.scalar.copy(bcs[:], psbc[:])
                            nc.vector.tensor_tensor(at[:, nq:nq + 512], psav[:],
                                                    bcs[:], op=ALU.mult)
                        attn_t.append(at)

                    # W_o partial for batch b's tokens (half-row output tiles)
                    for m in range(8):
                        for half in range(4):
                            os_ = opool.tile([128, 1024], F32, tag="os")
                            for n in range(2):
                                nf = half * 1024 + n * 512
                                pso = ps_o.tile([128, 512], F32, tag="o")
                                for l in range(HC):
                                    nc.tensor.matmul(
                                        pso[:], attn_t[l][:, m * 128:(m + 1) * 128],
                                        wo_a[:, l * H + nf:l * H + nf + 512],
                                        start=(l == 0), stop=(l == HC - 1))
                                if n % 2 == 0:
                                    nc.vector.tensor_copy(
                                        os_[:, n * 512:(n + 1) * 512], pso[:])
                                else:
                                    nc.scalar.copy(
                                        os_[:, n * 512:(n + 1) * 512], pso[:])
                            nc.sync.dma_start(
                                out_p[bs + m * 128:bs + (m + 1) * 128,
                                      half * 1024:(half + 1) * 1024], os_[:])
    nc.compile()
    return nc


def _host_prep(hidden_states, W_pack, W_o, attention_mask, position_ids):
    import ml_dtypes
    hidden_states = np.asarray(hidden_states, dtype=np.float32)
    W_pack = np.asarray(W_pack, dtype=np.float32)
    W_o = np.asarray(W_o, dtype=np.float32)
    attention_mask = np.asarray(attention_mask, dtype=np.float32)
    pos = np.asarray(position_ids)

    xT = np.ascontiguousarray(hidden_states.reshape(T, H).T)
    # exp(mask): softmax mask applied multiplicatively after exp
    maskT = np.ascontiguousarray(
        np.exp(attention_mask[:, 0].transpose(0, 2, 1)).astype(ml_dtypes.bfloat16))

    inv = (1.0 / (ROPE_BASE ** (np.arange(0, D, 2, dtype=np.float64) / D)))
    inv = np.concatenate([inv, inv])                       # [D]
    ang = pos.astype(np.float64).reshape(T)[None, :] * inv[:, None]   # [D, T]
    cosT = np.cos(ang).astype(np.float32)
    sinT = np.sin(ang).astype(np.float32)
    sinS = sinT.copy()
    sinS[:64] = -sinT[:64]
    cosT = np.ascontiguousarray(cosT)
    sinS = np.ascontiguousarray(sinS)

    in_maps = []
    for c in range(N_CORES):
        h0 = c * HC
        # wqk column order per pass p: [q_{2p}, k_{2p}, q_{2p+1}, k_{2p+1}]
        qcols = [W_pack[:, (h0 + l) * D:(h0 + l + 1) * D] for l in range(HC)]
        kcols = [W_pack[:, H + (h0 + l) * D:H + (h0 + l + 1) * D] for l in range(HC)]
        vcols = [W_pack[:, 2 * H + (h0 + l) * D:2 * H + (h0 + l + 1) * D]
                 for l in range(HC)]
        wqk_np = np.ascontiguousarray(np.concatenate(
            [qcols[0], kcols[0], qcols[1], kcols[1],
             qcols[2], kcols[2], qcols[3], kcols[3]], axis=1))
        wv_np = np.ascontiguousarray(np.concatenate(vcols, axis=1))
        wo_np = np.ascontiguousarray(W_o[h0 * D:(h0 + HC) * D, :])
        in_maps.append({
            "xT": xT, "wqk": wqk_np, "wv": wv_np, "wo": wo_np,
            "cosT": cosT, "sinS": sinS, "maskT": maskT,
        })
    return in_maps


def kernel(hidden_states, W_pack, W_o, attention_mask, position_ids):
    if "nc" not in _CACHE:
        _CACHE["nc"] = _build_module()
    nc = _CACHE["nc"]
    in_maps = _host_prep(hidden_states, W_pack, W_o, attention_mask, position_ids)
    res = bass_utils.run_bass_kernel_spmd(nc, in_maps, core_ids=list(range(N_CORES)))
    out = res.results[0]["out_p"].astype(np.float64)
    for c in range(1, N_CORES):
        out += res.results[c]["out_p"]
    return out.reshape(B, S, H).astype(np.float32)


# revision 27
# speedup vs baseline: 1.2474x; 1.0034x over previous
"""Trainium2 Bass kernel for nn_Attention_60567628808865.

Dense transformer attention block (B=4, S=1024, H=4096, NH=32, D=128):
  qkv = x @ W_pack; RoPE(q, k); causal-masked softmax attention; out @ W_o.

Sharding: tensor-parallel over heads across 8 NeuronCores. Each core computes
4 heads end-to-end (QKV projection with its W_pack column slice, attention,
and its W_o row-slice partial of the output projection); the host sums the 8
partial outputs.

All matmuls run in float32r (TF32) at full PE rate; accumulation is fp32 in
PSUM. Everything on-chip works in a transposed layout (features on the
partition axis) so no transposes are needed anywhere:
  qT/kT [d, t] <- lhsT=W_qk, rhs=xT      scoresT [tk, tq] <- lhsT=kT, rhs=qT
  v [t, d]     <- lhsT=xT,   rhs=W_v     attnT [d, tq]    <- lhsT=v,  rhs=expT
  out [t, f]   <- lhsT=attnT, rhs=W_o
Softmax runs unnormalized (no max-subtraction; scores are O(1) by
construction and exp(-1e9)=0), with the denominator computed by a ones-vector
matmul accumulated in PSUM and applied after PV via a K=1 broadcast matmul.
RoPE's rotate-half is a partition shift, done for free in the DMA that loads
q/k back from scratch, with the sign folded into the host-built sin table.
DMAs are batched into multi-dim-AP transfers (the HWDGE issue path costs
~625ns per DMA instruction, so many small DMAs throttle the PE).
"""
import numpy as np

import concourse.bass as bass  # noqa: F401  (AP types come via tile/bacc)
import concourse.tile as tile
from contextlib import ExitStack
from concourse import bacc, mybir
from concourse import bass_utils

F32 = mybir.dt.float32
F32R = mybir.dt.float32r
BF16 = mybir.dt.bfloat16
AF = mybir.ActivationFunctionType
ALU = mybir.AluOpType

B, S, H, NH = 4, 1024, 4096, 32
D = H // NH          # 128
T = B * S            # 4096 tokens
N_CORES = 8
HC = NH // N_CORES   # 4 heads per core
SCALE = float(1.0 / np.sqrt(D))
ROPE_BASE = 10000.0

TB = 256             # phase-1 token block (moving dim; >=256 keeps f32r at full rate)
NTB = T // TB        # 16
KT = H // 128        # 32 contraction tiles

_CACHE = {}


def _build_module(phases=("p1", "p2")):
    nc = bacc.Bacc("TRN2", target_bir_lowering=False, debug=False,
                   num_devices=N_CORES)

    xT = nc.dram_tensor("xT", [H, T], F32R, kind="ExternalInput").ap()
    wqk = nc.dram_tensor("wqk", [H, 2 * HC * D], F32R, kind="ExternalInput").ap()
    wv = nc.dram_tensor("wv", [H, HC * D], F32R, kind="ExternalInput").ap()
    wo = nc.dram_tensor("wo", [HC * D, H], F32R, kind="ExternalInput").ap()
    cosT = nc.dram_tensor("cosT", [D, T], F32, kind="ExternalInput").ap()
    sinS = nc.dram_tensor("sinS", [D, T], F32, kind="ExternalInput").ap()
    maskT = nc.dram_tensor("maskT", [B, S, S], BF16, kind="ExternalInput").ap()
    out_p = nc.dram_tensor("out_p", [T, H], F32, kind="ExternalOutput").ap()
    ones128 = nc.inline_tensor(np.ones((128, 1), np.float32), "ones128").ap().bitcast(F32R)
    ones1 = nc.inline_tensor(np.ones((1, 128), np.float32), "ones1").ap().bitcast(F32R)

    with tile.TileContext(nc) as tc, \
         nc.allow_low_precision(reason="tf32 matmuls; verified against reference"):
        with ExitStack() as octx:
            dram = octx.enter_context(tc.tile_pool(name="dram", bufs=1, space="DRAM"))
            cpool = octx.enter_context(tc.tile_pool(name="consts", bufs=1))
            # scratch: qkT rows (pass p, m): [q_2p, k_2p, q_2p+1, k_2p+1]
            qkT_d = dram.tile([2 * HC * D, T], F32R)
            v_d = dram.tile([T, HC * D], F32R)

            o128 = cpool.tile([128, 1], F32R)
            nc.sync.dma_start(o128[:], ones128[:])
            o1 = cpool.tile([1, 128], F32R)
            nc.sync.dma_start(o1[:], ones1[:])

            # ---------------- Phase 1: QKV projection ----------------
            if "p1" in phases:
              with ExitStack() as ctx:
                wpool = ctx.enter_context(tc.tile_pool(name="p1w", bufs=1))
                xpool = ctx.enter_context(tc.tile_pool(name="p1x", bufs=2))
                opool = ctx.enter_context(tc.tile_pool(name="p1o", bufs=2))
                cpool1 = ctx.enter_context(tc.tile_pool(name="p1cs", bufs=2))
                rpool1 = ctx.enter_context(tc.tile_pool(name="p1rope", bufs=2))
                pqk = ctx.enter_context(tc.tile_pool(name="p1pqk", bufs=6, space="PSUM"))
                pv = ctx.enter_context(tc.tile_pool(name="p1pv", bufs=2, space="PSUM"))

                for p in range(2):
                    # resident weights, split into half-K DMAs so the first
                    # accumulation chains can start at half-load
                    KH = KT // 2
                    wqk_a = wpool.tile([128, KT * 512], F32R, tag="wqk")
                    wv_a = wpool.tile([128, KT * 256], F32R, tag="wv")
                    for kh in range(2):
                        nc.sync.dma_start(
                            wqk_a[:, kh * KH * 512:(kh + 1) * KH * 512]
                                .rearrange("p (kk f) -> p kk f", kk=KH),
                            wqk[kh * KH * 128:(kh + 1) * KH * 128,
                                p * 512:(p + 1) * 512]
                                .rearrange("(kk p) f -> p kk f", p=128))
                        nc.sync.dma_start(
                            wv_a[:, kh * KH * 256:(kh + 1) * KH * 256]
                                .rearrange("p (kk f) -> p kk f", kk=KH),
                            wv[kh * KH * 128:(kh + 1) * KH * 128,
                               p * 256:(p + 1) * 256]
                                .rearrange("(kk p) f -> p kk f", p=128))

                    for tb in range(NTB):
                        t0 = tb * TB
                        cos_tb = cpool1.tile([128, TB], F32, tag="cos")
                        nc.sync.dma_start(cos_tb[:], cosT[:, t0:t0 + TB])
                        sin_tb = cpool1.tile([128, TB], F32, tag="sin")
                        nc.sync.dma_start(sin_tb[:], sinS[:, t0:t0 + TB])
                        xall = xpool.tile([128, KT * TB], F32R, tag="x")
                        for kh in range(2):
                            nc.sync.dma_start(
                                xall[:, kh * 16 * TB:(kh + 1) * 16 * TB]
                                    .rearrange("p (kk t) -> p kk t", kk=16),
                                xT[kh * 2048:(kh + 1) * 2048, t0:t0 + TB]
                                    .rearrange("(kk p) t -> p kk t", p=128))

                        qs_all = opool.tile([128, 4 * TB], F32R, tag="qs")
                        for m in range(4):
                            ps = pqk.tile([128, TB], F32, tag="qk")
                            for kk in range(KT):
                                nc.tensor.matmul(
                                    ps[:],
                                    wqk_a[:, kk * 512 + m * 128:kk * 512 + (m + 1) * 128],
                                    xall[:, kk * TB:(kk + 1) * TB],
                                    start=(kk == 0), stop=(kk == KT - 1))
                            # RoPE fused into the epilogue: rotate-half via
                            # partition-shifted copies, sign folded into sinS
                            rot = rpool1.tile([128, TB], F32, tag="rot")
                            nc.vector.tensor_copy(rot[0:64, :], ps[64:128, :])
                            nc.vector.tensor_copy(rot[64:128, :], ps[0:64, :])
                            m1_ = rpool1.tile([128, TB], F32, tag="m1")
                            nc.vector.tensor_tensor(m1_[:], ps[:], cos_tb[:],
                                                    op=ALU.mult)
                            m2_ = rpool1.tile([128, TB], F32, tag="m2")
                            nc.vector.tensor_tensor(m2_[:], rot[:], sin_tb[:],
                                                    op=ALU.mult)
                            nc.vector.tensor_tensor(qs_all[:, m * TB:(m + 1) * TB],
                                                    m1_[:], m2_[:], op=ALU.add)
                        # one DMA: rows (p*4+m)*128 for m in 0..4
                        nc.sync.dma_start(
                            qkT_d[p * 512:(p + 1) * 512, t0:t0 + TB]
                                .rearrange("(m pp) t -> pp m t", pp=128),
                            qs_all[:].rearrange("pp (m t) -> pp m t", m=4))

                        vs_all = opool.tile([128, 2 * 256], F32R, tag="vs")
                        for mt in range(2):
                            ps = pv.tile([128, 256], F32, tag="v")
                            for kk in range(KT):
                                nc.tensor.matmul(
                                    ps[:],
                                    xall[:, kk * TB + mt * 128:kk * TB + (mt + 1) * 128],
                                    wv_a[:, kk * 256:(kk + 1) * 256],
                                    start=(kk == 0), stop=(kk == KT - 1))
                            nc.vector.tensor_copy(vs_all[:, mt * 256:(mt + 1) * 256], ps[:])
                        nc.sync.dma_start(
                            v_d[t0:t0 + TB, p * 256:(p + 1) * 256]
                                .rearrange("(mt pp) f -> pp mt f", pp=128),
                            vs_all[:].rearrange("pp (mt f) -> pp mt f", mt=2))

            # ---------------- Phase 2+3: attention + W_o ----------------
            if "p2" in phases:
              with ExitStack() as ctx:
                wopool = ctx.enter_context(tc.tile_pool(name="p2wo", bufs=1))
                mpool = ctx.enter_context(tc.tile_pool(name="p2m", bufs=1))
                m2pool = ctx.enter_context(tc.tile_pool(name="p2m2", bufs=2))
                tpool = ctx.enter_context(tc.tile_pool(name="p2t", bufs=2))
                epool = ctx.enter_context(tc.tile_pool(name="p2e", bufs=3))
                efpool = ctx.enter_context(tc.tile_pool(name="p2ef", bufs=10))
                apool = ctx.enter_context(tc.tile_pool(name="p2a", bufs=2))
                opool = ctx.enter_context(tc.tile_pool(name="p2o", bufs=2))
                ps_s = ctx.enter_context(tc.tile_pool(name="p2ps", bufs=2, space="PSUM"))
                ps_o = ctx.enter_context(tc.tile_pool(name="p2po", bufs=2, space="PSUM"))
                ps_bc = ctx.enter_context(tc.tile_pool(name="p2pbc", bufs=1, space="PSUM"))
                ps_d = ctx.enter_context(tc.tile_pool(name="p2pd", bufs=1, space="PSUM"))
                ps_av = ctx.enter_context(tc.tile_pool(name="p2pav", bufs=2, space="PSUM"))

                # W_o resident: one tile; DMA issued after the first head's
                # input loads so attention starts immediately
                wo_a = wopool.tile([128, HC * H], F32R, tag="wo")
                wo_loaded = [False]

                def load_wo():
                    if not wo_loaded[0]:
                        nc.sync.dma_start(
                            wo_a[:].rearrange("p (l f) -> p l f", l=HC),
                            wo.rearrange("(l p) f -> p l f", p=128))
                        wo_loaded[0] = True

                for b in range(B):
                    bs = b * S
                    mask_a = m2pool.tile([128, 4 * S], BF16, tag="maskA")
                    nc.sync.dma_start(
                        mask_a[:].rearrange("p (mt t) -> p mt t", mt=4),
                        maskT[b, 0:512].rearrange("(mt p) t -> p mt t", p=128))
                    mask_bb = m2pool.tile([128, 4 * S], BF16, tag="maskB")
                    nc.sync.dma_start(
                        mask_bb[:].rearrange("p (mt t) -> p mt t", mt=4),
                        maskT[b, 512:1024].rearrange("(mt p) t -> p mt t", p=128))
                    mask_halves = (mask_a, mask_bb)

                    attn_t = []
                    for l in range(HC):
                        rq = (4 * (l // 2) + 2 * (l % 2)) * 128
                        vcol = (l // 2) * 256 + (l % 2) * 128

                        # rope'd q,k load: [p, j(q/k), t] (1 DMA)
                        kq = tpool.tile([128, 2 * S], F32R, tag="kqraw")
                        nc.sync.dma_start(
                            kq[:].rearrange("p (j t) -> p j t", j=2),
                            qkT_d[rq:rq + 256, bs:bs + S]
                                .rearrange("(j p) t -> p j t", p=128))
                        vt_ = tpool.tile([128, 8 * 128], F32R, tag="vt")
                        nc.sync.dma_start(
                            vt_[:].rearrange("p (kt d) -> p kt d", kt=8),
                            v_d[bs:bs + S, vcol:vcol + 128]
                               .rearrange("(kt p) d -> p kt d", p=128))
                        q_rope = kq[:, 0:S]
                        k_rope = kq[:, S:2 * S]
                        load_wo()

                        at = apool.tile([128, S], F32R, tag=f"attn{l}")
                        for nt in range(2):
                            nq = nt * 512
                            psd = ps_d.tile([1, 512], F32, tag="d")
                            psav = ps_av.tile([128, 512], F32, tag="av")
                            ef_tiles = []
                            for mt in range(8):
                                pss = ps_s.tile([128, 512], F32, tag="s")
                                nc.tensor.matmul(
                                    pss[:], k_rope[:, mt * 128:(mt + 1) * 128],
                                    q_rope[:, nq:nq + 512], start=True, stop=True)
                                ef0 = epool.tile([128, 512], F32, tag="ef0")
                                nc.scalar.activation(ef0[:], pss[:], AF.Exp,
                                                     scale=SCALE)
                                ef = efpool.tile([128, 512], F32R, tag="ef")
                                mh = mask_halves[mt // 4]
                                msl = mh[:, (mt % 4) * S + nq:(mt % 4) * S + nq + 512]
                                eng = nc.gpsimd if mt % 4 == 3 else nc.vector
                                eng.tensor_tensor(ef[:], ef0[:], msl, op=ALU.mult)
                                ef_tiles.append(ef)
                                nc.tensor.matmul(
                                    psav[:], vt_[:, mt * 128:(mt + 1) * 128], ef[:],
                                    start=(mt == 0), stop=(mt == 7))
                            for mt in range(8):
                                nc.tensor.matmul(psd[:], o128[:], ef_tiles[mt][:],
                                                 start=(mt == 0), stop=(mt == 7))
                            rd = epool.tile([1, 512], F32R, tag="rd")
                            nc.vector.reciprocal(rd[:], psd[:])
                            psbc = ps_bc.tile([128, 512], F32, tag="bc")
                            nc.tensor.matmul(psbc[:], o1[:], rd[:], start=True, stop=True)
                            bcs = epool.tile([128, 512], F32, tag="bcs")
                            nc.scalar.copy(bcs[:], psbc[:])
                            nc.vector.tensor_tensor(at[:, nq:nq + 512], psav[:],
                                                    bcs[:], op=ALU.mult)
                        attn_t.append(at)

                    # W_o partial for batch b's tokens (half-row output tiles)
                    for m in range(8):
                        for half in range(4):
                            os_ = opool.tile([128, 1024], F32, tag="os")
                            for n in range(2):
                                nf = half * 1024 + n * 512
                                pso = ps_o.tile([128, 512], F32, tag="o")
                                for l in range(HC):
                                    nc.tensor.matmul(
                                        pso[:], attn_t[l][:, m * 128:(m + 1) * 128],
                                        wo_a[:, l * H + nf:l * H + nf + 512],
                                        start=(l == 0), stop=(l == HC - 1))
                                if n % 2 == 0:
                                    nc.vector.tensor_copy(
                                        os_[:, n * 512:(n + 1) * 512], pso[:])
                                else:
                                    nc.scalar.copy(
                                        os_[:, n * 512:(n + 1) * 512], pso[:])
                            nc.sync.dma_start(
                                out_p[bs + m * 128:bs + (m + 1) * 128,
                                      half * 1024:(half + 1) * 1024], os_[:])
    nc.compile()
    return nc


def _host_prep(hidden_states, W_pack, W_o, attention_mask, position_ids):
    import ml_dtypes
    hidden_states = np.asarray(hidden_states, dtype=np.float32)
    W_pack = np.asarray(W_pack, dtype=np.float32)
    W_o = np.asarray(W_o, dtype=np.float32)
    attention_mask = np.asarray(attention_mask, dtype=np.float32)
    pos = np.asarray(position_ids)

    xT = np.ascontiguousarray(hidden_states.reshape(T, H).T)
    # exp(mask): softmax mask applied multiplicatively after exp
    maskT = np.ascontiguousarray(
        np.exp(attention_mask[:, 0].transpose(0, 2, 1)).astype(ml_dtypes.bfloat16))

    inv = (1.0 / (ROPE_BASE ** (np.arange(0, D, 2, dtype=np.float64) / D)))
    inv = np.concatenate([inv, inv])                       # [D]
    ang = pos.astype(np.float64).reshape(T)[None, :] * inv[:, None]   # [D, T]
    cosT = np.cos(ang).astype(np.float32)
    sinT = np.sin(ang).astype(np.float32)
    sinS = sinT.copy()
    sinS[:64] = -sinT[:64]
    cosT = np.ascontiguousarray(cosT)
    sinS = np.ascontiguousarray(sinS)

    in_maps = []
    for c in range(N_CORES):
        h0 = c * HC
        # wqk column order per pass p: [q_{2p}, k_{2p}, q_{2p+1}, k_{2p+1}]
        qcols = [W_pack[:, (h0 + l) * D:(h0 + l + 1) * D] for l in range(HC)]
        kcols = [W_pack[:, H + (h0 + l) * D:H + (h0 + l + 1) * D] for l in range(HC)]
        vcols = [W_pack[:, 2 * H + (h0 + l) * D:2 * H + (h0 + l + 1) * D]
                 for l in range(HC)]
        wqk_np = np.ascontiguousarray(np.concatenate(
            [qcols[0], kcols[0], qcols[1], kcols[1],
             qcols[2], kcols[2], qcols[3], kcols[3]], axis=1))
        wv_np = np.ascontiguousarray(np.concatenate(vcols, axis=1))
        wo_np = np.ascontiguousarray(W_o[h0 * D:(h0 + HC) * D, :])
        in_maps.append({
            "xT": xT, "wqk": wqk_np, "wv": wv_np, "wo": wo_np,
            "cosT": cosT, "sinS": sinS, "maskT": maskT,
        })
    return in_maps


def kernel(hidden_states, W_pack, W_o, attention_mask, position_ids):
    if "nc" not in _CACHE:
        _CACHE["nc"] = _build_module()
    nc = _CACHE["nc"]
    in_maps = _host_prep(hidden_states, W_pack, W_o, attention_mask, position_ids)
    res = bass_utils.run_bass_kernel_spmd(nc, in_maps, core_ids=list(range(N_CORES)))
    out = res.results[0]["out_p"].astype(np.float64)
    for c in range(1, N_CORES):
        out += res.results[c]["out_p"]
    return out.reshape(B, S, H).astype(np.float32)


# revision 32
# speedup vs baseline: 1.2491x; 1.0014x over previous
"""Trainium2 Bass kernel for nn_Attention_60567628808865.

Dense transformer attention block (B=4, S=1024, H=4096, NH=32, D=128):
  qkv = x @ W_pack; RoPE(q, k); causal-masked softmax attention; out @ W_o.

Sharding: tensor-parallel over heads across 8 NeuronCores. Each core computes
4 heads end-to-end (QKV projection with its W_pack column slice, attention,
and its W_o row-slice partial of the output projection); the host sums the 8
partial outputs.

All matmuls run in float32r (TF32) at full PE rate; accumulation is fp32 in
PSUM. Everything on-chip works in a transposed layout (features on the
partition axis) so no transposes are needed anywhere:
  qT/kT [d, t] <- lhsT=W_qk, rhs=xT      scoresT [tk, tq] <- lhsT=kT, rhs=qT
  v [t, d]     <- lhsT=xT,   rhs=W_v     attnT [d, tq]    <- lhsT=v,  rhs=expT
  out [t, f]   <- lhsT=attnT, rhs=W_o
Softmax runs unnormalized (no max-subtraction; scores are O(1) by
construction and exp(-1e9)=0), with the denominator computed by a ones-vector
matmul accumulated in PSUM and applied after PV via a K=1 broadcast matmul.
RoPE's rotate-half is a partition shift, done for free in the DMA that loads
q/k back from scratch, with the sign folded into the host-built sin table.
DMAs are batched into multi-dim-AP transfers (the HWDGE issue path costs
~625ns per DMA instruction, so many small DMAs throttle the PE).
"""
import numpy as np

import concourse.bass as bass  # noqa: F401  (AP types come via tile/bacc)
import concourse.tile as tile
from contextlib import ExitStack
from concourse import bacc, mybir
from concourse import bass_utils

F32 = mybir.dt.float32
F32R = mybir.dt.float32r
BF16 = mybir.dt.bfloat16
AF = mybir.ActivationFunctionType
ALU = mybir.AluOpType

B, S, H, NH = 4, 1024, 4096, 32
D = H // NH          # 128
T = B * S            # 4096 tokens
N_CORES = 8
HC = NH // N_CORES   # 4 heads per core
SCALE = float(1.0 / np.sqrt(D))
ROPE_BASE = 10000.0

TB = 256             # phase-1 token block (moving dim; >=256 keeps f32r at full rate)
NTB = T // TB        # 16
KT = H // 128        # 32 contraction tiles

_CACHE = {}


def _build_module(phases=("p1", "p2")):
    nc = bacc.Bacc("TRN2", target_bir_lowering=False, debug=False,
                   num_devices=N_CORES)

    xT = nc.dram_tensor("xT", [H, T], F32R, kind="ExternalInput").ap()
    wqk = nc.dram_tensor("wqk", [H, 2 * HC * D], F32R, kind="ExternalInput").ap()
    wv = nc.dram_tensor("wv", [H, HC * D], F32R, kind="ExternalInput").ap()
    wo = nc.dram_tensor("wo", [HC * D, H], F32R, kind="ExternalInput").ap()
    cosT = nc.dram_tensor("cosT", [D, T], F32, kind="ExternalInput").ap()
    sinS = nc.dram_tensor("sinS", [D, T], F32, kind="ExternalInput").ap()
    maskT = nc.dram_tensor("maskT", [B, S, S], BF16, kind="ExternalInput").ap()
    out_p = nc.dram_tensor("out_p", [T, H], F32, kind="ExternalOutput").ap()
    ones128 = nc.inline_tensor(np.ones((128, 1), np.float32), "ones128").ap().bitcast(F32R)
    ones1 = nc.inline_tensor(np.ones((1, 128), np.float32), "ones1").ap().bitcast(F32R)

    with tile.TileContext(nc) as tc, \
         nc.allow_low_precision(reason="tf32 matmuls; verified against reference"):
        with ExitStack() as octx:
            dram = octx.enter_context(tc.tile_pool(name="dram", bufs=1, space="DRAM"))
            cpool = octx.enter_context(tc.tile_pool(name="consts", bufs=1))
            # scratch: qkT rows (pass p, m): [q_2p, k_2p, q_2p+1, k_2p+1]
            qkT_d = dram.tile([2 * HC * D, T], F32R)
            v_d = dram.tile([T, HC * D], F32R)

            o128 = cpool.tile([128, 1], F32R)
            nc.sync.dma_start(o128[:], ones128[:])
            o1 = cpool.tile([1, 128], F32R)
            nc.sync.dma_start(o1[:], ones1[:])

            # ---------------- Phase 1: QKV projection ----------------
            if "p1" in phases:
              with ExitStack() as ctx:
                wpool = ctx.enter_context(tc.tile_pool(name="p1w", bufs=1))
                xpool = ctx.enter_context(tc.tile_pool(name="p1x", bufs=2))
                opool = ctx.enter_context(tc.tile_pool(name="p1o", bufs=2))
                cpool1 = ctx.enter_context(tc.tile_pool(name="p1cs", bufs=2))
                rpool1 = ctx.enter_context(tc.tile_pool(name="p1rope", bufs=2))
                pqk = ctx.enter_context(tc.tile_pool(name="p1pqk", bufs=6, space="PSUM"))
                pv = ctx.enter_context(tc.tile_pool(name="p1pv", bufs=2, space="PSUM"))

                last_x = [None]
                for p in range(2):
                    # resident weights, split into half-K DMAs so the first
                    # accumulation chains can start at half-load
                    KH = KT // 2
                    wqk_a = wpool.tile([128, KT * 512], F32R, tag="wqk")
                    wv_a = wpool.tile([128, KT * 256], F32R, tag="wv")
                    for kh in range(2):
                        nc.sync.dma_start(
                            wqk_a[:, kh * KH * 512:(kh + 1) * KH * 512]
                                .rearrange("p (kk f) -> p kk f", kk=KH),
                            wqk[kh * KH * 128:(kh + 1) * KH * 128,
                                p * 512:(p + 1) * 512]
                                .rearrange("(kk p) f -> p kk f", p=128))
                    for kh in range(2):
                        nc.sync.dma_start(
                            wv_a[:, kh * KH * 256:(kh + 1) * KH * 256]
                                .rearrange("p (kk f) -> p kk f", kk=KH),
                            wv[kh * KH * 128:(kh + 1) * KH * 128,
                               p * 256:(p + 1) * 256]
                                .rearrange("(kk p) f -> p kk f", p=128))

                    tb_order = range(NTB) if p == 0 else \
                        list(reversed(range(NTB)))
                    for tb in tb_order:
                        t0 = tb * TB
                        cos_tb = cpool1.tile([128, TB], F32, tag="cos")
                        nc.sync.dma_start(cos_tb[:], cosT[:, t0:t0 + TB])
                        sin_tb = cpool1.tile([128, TB], F32, tag="sin")
                        nc.sync.dma_start(sin_tb[:], sinS[:, t0:t0 + TB])
                        if p == 1 and tb == NTB - 1 and last_x[0] is not None:
                            xall = last_x[0]
                        else:
                            xall = xpool.tile([128, KT * TB], F32R, tag="x")
                            for kh in range(2):
                                nc.sync.dma_start(
                                    xall[:, kh * 16 * TB:(kh + 1) * 16 * TB]
                                        .rearrange("p (kk t) -> p kk t", kk=16),
                                    xT[kh * 2048:(kh + 1) * 2048, t0:t0 + TB]
                                        .rearrange("(kk p) t -> p kk t", p=128))
                        if p == 0 and tb == NTB - 1:
                            last_x[0] = xall

                        qs_all = opool.tile([128, 4 * TB], F32R, tag="qs")
                        for m in range(4):
                            ps = pqk.tile([128, TB], F32, tag="qk")
                            for kk in range(KT):
                                nc.tensor.matmul(
                                    ps[:],
                                    wqk_a[:, kk * 512 + m * 128:kk * 512 + (m + 1) * 128],
                                    xall[:, kk * TB:(kk + 1) * TB],
                                    start=(kk == 0), stop=(kk == KT - 1))
                            # RoPE fused into the epilogue: rotate-half via
                            # partition-shifted copies, sign folded into sinS
                            rot = rpool1.tile([128, TB], F32, tag="rot")
                            nc.vector.tensor_copy(rot[0:64, :], ps[64:128, :])
                            nc.vector.tensor_copy(rot[64:128, :], ps[0:64, :])
                            m1_ = rpool1.tile([128, TB], F32, tag="m1")
                            nc.vector.tensor_tensor(m1_[:], ps[:], cos_tb[:],
                                                    op=ALU.mult)
                            m2_ = rpool1.tile([128, TB], F32, tag="m2")
                            nc.vector.tensor_tensor(m2_[:], rot[:], sin_tb[:],
                                                    op=ALU.mult)
                            nc.vector.tensor_tensor(qs_all[:, m * TB:(m + 1) * TB],
                                                    m1_[:], m2_[:], op=ALU.add)
                        # one DMA: rows (p*4+m)*128 for m in 0..4
                        nc.sync.dma_start(
                            qkT_d[p * 512:(p + 1) * 512, t0:t0 + TB]
                                .rearrange("(m pp) t -> pp m t", pp=128),
                            qs_all[:].rearrange("pp (m t) -> pp m t", m=4))

                        vs_all = opool.tile([128, 2 * 256], F32R, tag="vs")
                        for mt in range(2):
                            ps = pv.tile([128, 256], F32, tag="v")
                            for kk in range(KT):
                                nc.tensor.matmul(
                                    ps[:],
                                    xall[:, kk * TB + mt * 128:kk * TB + (mt + 1) * 128],
                                    wv_a[:, kk * 256:(kk + 1) * 256],
                                    start=(kk == 0), stop=(kk == KT - 1))
                            nc.vector.tensor_copy(vs_all[:, mt * 256:(mt + 1) * 256], ps[:])
                        nc.sync.dma_start(
                            v_d[t0:t0 + TB, p * 256:(p + 1) * 256]
                                .rearrange("(mt pp) f -> pp mt f", pp=128),
                            vs_all[:].rearrange("pp (mt f) -> pp mt f", mt=2))

            # ---------------- Phase 2+3: attention + W_o ----------------
            if "p2" in phases:
              with ExitStack() as ctx:
                wopool = ctx.enter_context(tc.tile_pool(name="p2wo", bufs=1))
                mpool = ctx.enter_context(tc.tile_pool(name="p2m", bufs=1))
                m2pool = ctx.enter_context(tc.tile_pool(name="p2m2", bufs=2))
                tpool = ctx.enter_context(tc.tile_pool(name="p2t", bufs=2))
                epool = ctx.enter_context(tc.tile_pool(name="p2e", bufs=3))
                efpool = ctx.enter_context(tc.tile_pool(name="p2ef", bufs=10))
                apool = ctx.enter_context(tc.tile_pool(name="p2a", bufs=2))
                opool = ctx.enter_context(tc.tile_pool(name="p2o", bufs=2))
                ps_s = ctx.enter_context(tc.tile_pool(name="p2ps", bufs=2, space="PSUM"))
                ps_o = ctx.enter_context(tc.tile_pool(name="p2po", bufs=2, space="PSUM"))
                ps_bc = ctx.enter_context(tc.tile_pool(name="p2pbc", bufs=1, space="PSUM"))
                ps_d = ctx.enter_context(tc.tile_pool(name="p2pd", bufs=1, space="PSUM"))
                ps_av = ctx.enter_context(tc.tile_pool(name="p2pav", bufs=2, space="PSUM"))

                # W_o resident: one tile; DMA issued after the first head's
                # input loads so attention starts immediately
                wo_a = wopool.tile([128, HC * H], F32R, tag="wo")
                wo_loaded = [False]

                def load_wo():
                    if not wo_loaded[0]:
                        nc.sync.dma_start(
                            wo_a[:].rearrange("p (l f) -> p l f", l=HC),
                            wo.rearrange("(l p) f -> p l f", p=128))
                        wo_loaded[0] = True

                for b in range(B):
                    bs = b * S
                    mask_a = m2pool.tile([128, 4 * S], BF16, tag="maskA")
                    nc.sync.dma_start(
                        mask_a[:].rearrange("p (mt t) -> p mt t", mt=4),
                        maskT[b, 0:512].rearrange("(mt p) t -> p mt t", p=128))
                    mask_bb = m2pool.tile([128, 4 * S], BF16, tag="maskB")
                    nc.sync.dma_start(
                        mask_bb[:].rearrange("p (mt t) -> p mt t", mt=4),
                        maskT[b, 512:1024].rearrange("(mt p) t -> p mt t", p=128))
                    mask_halves = (mask_a, mask_bb)

                    attn_t = []
                    for l in range(HC):
                        rq = (4 * (l // 2) + 2 * (l % 2)) * 128
                        vcol = (l // 2) * 256 + (l % 2) * 128

                        # rope'd q,k load: [p, j(q/k), t] (1 DMA)
                        kq = tpool.tile([128, 2 * S], F32R, tag="kqraw")
                        nc.sync.dma_start(
                            kq[:].rearrange("p (j t) -> p j t", j=2),
                            qkT_d[rq:rq + 256, bs:bs + S]
                                .rearrange("(j p) t -> p j t", p=128))
                        vt_ = tpool.tile([128, 8 * 128], F32R, tag="vt")
                        nc.sync.dma_start(
                            vt_[:].rearrange("p (kt d) -> p kt d", kt=8),
                            v_d[bs:bs + S, vcol:vcol + 128]
                               .rearrange("(kt p) d -> p kt d", p=128))
                        q_rope = kq[:, 0:S]
                        k_rope = kq[:, S:2 * S]
                        load_wo()

                        at = apool.tile([128, S], F32R, tag=f"attn{l}")
                        for nt in range(2):
                            nq = nt * 512
                            psd = ps_d.tile([1, 512], F32, tag="d")
                            psav = ps_av.tile([128, 512], F32, tag="av")
                            ef_tiles = []
                            for mt in range(8):
                                pss = ps_s.tile([128, 512], F32, tag="s")
                                nc.tensor.matmul(
                                    pss[:], k_rope[:, mt * 128:(mt + 1) * 128],
                                    q_rope[:, nq:nq + 512], start=True, stop=True)
                                ef0 = epool.tile([128, 512], F32, tag="ef0")
                                nc.scalar.activation(ef0[:], pss[:], AF.Exp,
                                                     scale=SCALE)
                                ef = efpool.tile([128, 512], F32R, tag="ef")
                                mh = mask_halves[mt // 4]
                                msl = mh[:, (mt % 4) * S + nq:(mt % 4) * S + nq + 512]
                                eng = nc.gpsimd if mt % 4 == 3 else nc.vector
                                eng.tensor_tensor(ef[:], ef0[:], msl, op=ALU.mult)
                                ef_tiles.append(ef)
                                nc.tensor.matmul(
                                    psav[:], vt_[:, mt * 128:(mt + 1) * 128], ef[:],
                                    start=(mt == 0), stop=(mt == 7))
                            for mt in range(8):
                                nc.tensor.matmul(psd[:], o128[:], ef_tiles[mt][:],
                                                 start=(mt == 0), stop=(mt == 7))
                            rd = epool.tile([1, 512], F32R, tag="rd")
                            nc.vector.reciprocal(rd[:], psd[:])
                            psbc = ps_bc.tile([128, 512], F32, tag="bc")
                            nc.tensor.matmul(psbc[:], o1[:], rd[:], start=True, stop=True)
                            bcs = epool.tile([128, 512], F32, tag="bcs")
                            nc.scalar.copy(bcs[:], psbc[:])
                            nc.vector.tensor_tensor(at[:, nq:nq + 512], psav[:],
                                                    bcs[:], op=ALU.mult)
                        attn_t.append(at)

                    # W_o partial for batch b's tokens (half-row output tiles)
                    for m in range(8):
                        for half in range(4):
                            os_ = opool.tile([128, 1024], F32, tag="os")
                            for n in range(2):
                                nf = half * 1024 + n * 512
                                pso = ps_o.tile([128, 512], F32, tag="o")
                                for l in range(HC):
                                    nc.tensor.matmul(
                                        pso[:], attn_t[l][:, m * 128:(m + 1) * 128],
                                        wo_a[:, l * H + nf:l * H + nf + 512],
                                        start=(l == 0), stop=(l == HC - 1))
                                if n % 2 == 0:
                                    nc.vector.tensor_copy(
                                        os_[:, n * 512:(n + 1) * 512], pso[:])
                                else:
                                    nc.scalar.copy(
                                        os_[:, n * 512:(n + 1) * 512], pso[:])
                            nc.sync.dma_start(
                                out_p[bs + m * 128:bs + (m + 1) * 128,
                                      half * 1024:(half + 1) * 1024], os_[:])
    nc.compile()
    return nc


def _host_prep(hidden_states, W_pack, W_o, attention_mask, position_ids):
    import ml_dtypes
    hidden_states = np.asarray(hidden_states, dtype=np.float32)
    W_pack = np.asarray(W_pack, dtype=np.float32)
    W_o = np.asarray(W_o, dtype=np.float32)
    attention_mask = np.asarray(attention_mask, dtype=np.float32)
    pos = np.asarray(position_ids)

    xT = np.ascontiguousarray(hidden_states.reshape(T, H).T)
    # exp(mask): softmax mask applied multiplicatively after exp
    maskT = np.ascontiguousarray(
        np.exp(attention_mask[:, 0].transpose(0, 2, 1)).astype(ml_dtypes.bfloat16))

    inv = (1.0 / (ROPE_BASE ** (np.arange(0, D, 2, dtype=np.float64) / D)))
    inv = np.concatenate([inv, inv])                       # [D]
    ang = pos.astype(np.float64).reshape(T)[None, :] * inv[:, None]   # [D, T]
    cosT = np.cos(ang).astype(np.float32)
    sinT = np.sin(ang).astype(np.float32)
    sinS = sinT.copy()
    sinS[:64] = -sinT[:64]
    cosT = np.ascontiguousarray(cosT)
    sinS = np.ascontiguousarray(sinS)

    in_maps = []
    for c in range(N_CORES):
        h0 = c * HC
        # wqk column order per pass p: [q_{2p}, k_{2p}, q_{2p+1}, k_{2p+1}]
        qcols = [W_pack[:, (h0 + l) * D:(h0 + l + 1) * D] for l in range(HC)]
        kcols = [W_pack[:, H + (h0 + l) * D:H + (h0 + l + 1) * D] for l in range(HC)]
        vcols = [W_pack[:, 2 * H + (h0 + l) * D:2 * H + (h0 + l + 1) * D]
                 for l in range(HC)]
        wqk_np = np.ascontiguousarray(np.concatenate(
            [qcols[0], kcols[0], qcols[1], kcols[1],
             qcols[2], kcols[2], qcols[3], kcols[3]], axis=1))
        wv_np = np.ascontiguousarray(np.concatenate(vcols, axis=1))
        wo_np = np.ascontiguousarray(W_o[h0 * D:(h0 + HC) * D, :])
        in_maps.append({
            "xT": xT, "wqk": wqk_np, "wv": wv_np, "wo": wo_np,
            "cosT": cosT, "sinS": sinS, "maskT": maskT,
        })
    return in_maps


def kernel(hidden_states, W_pack, W_o, attention_mask, position_ids):
    if "nc" not in _CACHE:
        _CACHE["nc"] = _build_module()
    nc = _CACHE["nc"]
    in_maps = _host_prep(hidden_states, W_pack, W_o, attention_mask, position_ids)
    res = bass_utils.run_bass_kernel_spmd(nc, in_maps, core_ids=list(range(N_CORES)))
    out = res.results[0]["out_p"].astype(np.float64)
    for c in range(1, N_CORES):
        out += res.results[c]["out_p"]
    return out.reshape(B, S, H).astype(np.float32)


# revision 42
# speedup vs baseline: 1.3101x; 1.0488x over previous
"""Trainium2 Bass kernel for nn_Attention_60567628808865.

Dense transformer attention block (B=4, S=1024, H=4096, NH=32, D=128):
  qkv = x @ W_pack; RoPE(q, k); causal-masked softmax attention; out @ W_o.

Sharding: tensor-parallel over heads across 8 NeuronCores. Each core computes
4 heads end-to-end (QKV projection with its W_pack column slice, attention,
and its W_o row-slice partial of the output projection); the host sums the 8
partial outputs.

All matmuls run in float32r (TF32) at full PE rate; accumulation is fp32 in
PSUM. Everything on-chip works in a transposed layout (features on the
partition axis) so no transposes are needed anywhere:
  qT/kT [d, t] <- lhsT=W_qk, rhs=xT      scoresT [tk, tq] <- lhsT=kT, rhs=qT
  v [t, d]     <- lhsT=xT,   rhs=W_v     attnT [d, tq]    <- lhsT=v,  rhs=expT
  out [t, f]   <- lhsT=attnT, rhs=W_o
Softmax runs unnormalized (no max-subtraction; scores are O(1) by
construction and exp(-1e9)=0), with the denominator computed by a ones-vector
matmul accumulated in PSUM and applied after PV via a K=1 broadcast matmul.
RoPE's rotate-half is a partition shift, done for free in the DMA that loads
q/k back from scratch, with the sign folded into the host-built sin table.
DMAs are batched into multi-dim-AP transfers (the HWDGE issue path costs
~625ns per DMA instruction, so many small DMAs throttle the PE).
"""
import numpy as np

import concourse.bass as bass  # noqa: F401  (AP types come via tile/bacc)
import concourse.tile as tile
from contextlib import ExitStack
from concourse import bacc, mybir
from concourse import bass_utils

F32 = mybir.dt.float32
F32R = mybir.dt.float32r
BF16 = mybir.dt.bfloat16
AF = mybir.ActivationFunctionType
ALU = mybir.AluOpType

B, S, H, NH = 4, 1024, 4096, 32
D = H // NH          # 128
T = B * S            # 4096 tokens
N_CORES = 8
HC = NH // N_CORES   # 4 heads per core
SCALE = float(1.0 / np.sqrt(D))
ROPE_BASE = 10000.0

TB = 256             # phase-1 token block (moving dim; >=256 keeps f32r at full rate)
NTB = T // TB        # 16
KT = H // 128        # 32 contraction tiles

_CACHE = {}


def _build_module(phases=("p1", "p2")):
    nc = bacc.Bacc("TRN2", target_bir_lowering=False, debug=False,
                   num_devices=N_CORES)

    xT = nc.dram_tensor("xT", [H, T], F32R, kind="ExternalInput").ap()
    wqk = nc.dram_tensor("wqk", [H, 2 * HC * D], F32R, kind="ExternalInput").ap()
    wv = nc.dram_tensor("wv", [H, HC * D], F32R, kind="ExternalInput").ap()
    wo = nc.dram_tensor("wo", [HC * D, H], F32R, kind="ExternalInput").ap()
    cosT = nc.dram_tensor("cosT", [D, T], F32, kind="ExternalInput").ap()
    sinS = nc.dram_tensor("sinS", [D, T], F32, kind="ExternalInput").ap()
    maskT = nc.dram_tensor("maskT", [B, S, S], BF16, kind="ExternalInput").ap()
    out_p = nc.dram_tensor("out_p", [T, H], F32, kind="ExternalOutput").ap()
    ones128 = nc.inline_tensor(np.ones((128, 1), np.float32), "ones128").ap().bitcast(F32R)
    ones1 = nc.inline_tensor(np.ones((1, 128), np.float32), "ones1").ap().bitcast(F32R)

    with tile.TileContext(nc) as tc, \
         nc.allow_low_precision(reason="tf32 matmuls; verified against reference"):
        with ExitStack() as octx:
            dram = octx.enter_context(tc.tile_pool(name="dram", bufs=1, space="DRAM"))
            cpool = octx.enter_context(tc.tile_pool(name="consts", bufs=1))
            # scratch: qkT rows (pass p, m): [q_2p, k_2p, q_2p+1, k_2p+1]
            qkT_d = dram.tile([2 * HC * D, T], F32R)
            v_d = dram.tile([T, HC * D], F32R)

            o128 = cpool.tile([128, 1], F32R)
            nc.sync.dma_start(o128[:], ones128[:])
            o1 = cpool.tile([1, 128], F32R)
            nc.sync.dma_start(o1[:], ones1[:])

            # ---------------- Phase 1: QKV projection ----------------
            if "p1" in phases:
              with ExitStack() as ctx:
                wpool = ctx.enter_context(tc.tile_pool(name="p1w", bufs=1))
                xpool = ctx.enter_context(tc.tile_pool(name="p1x", bufs=2))
                opool = ctx.enter_context(tc.tile_pool(name="p1o", bufs=2))
                cpool1 = ctx.enter_context(tc.tile_pool(name="p1cs", bufs=2))
                rpool1 = ctx.enter_context(tc.tile_pool(name="p1rope", bufs=2))
                pqk = ctx.enter_context(tc.tile_pool(name="p1pqk", bufs=6, space="PSUM"))
                pv = ctx.enter_context(tc.tile_pool(name="p1pv", bufs=2, space="PSUM"))

                last_x = [None]
                for p in range(2):
                    # resident weights, split into half-K DMAs so the first
                    # accumulation chains can start at half-load
                    KH = KT // 2
                    wqk_a = wpool.tile([128, KT * 512], F32R, tag="wqk")
                    wv_a = wpool.tile([128, KT * 256], F32R, tag="wv")
                    for kh in range(2):
                        nc.sync.dma_start(
                            wqk_a[:, kh * KH * 512:(kh + 1) * KH * 512]
                                .rearrange("p (kk f) -> p kk f", kk=KH),
                            wqk[kh * KH * 128:(kh + 1) * KH * 128,
                                p * 512:(p + 1) * 512]
                                .rearrange("(kk p) f -> p kk f", p=128))
                    for kh in range(2):
                        nc.sync.dma_start(
                            wv_a[:, kh * KH * 256:(kh + 1) * KH * 256]
                                .rearrange("p (kk f) -> p kk f", kk=KH),
                            wv[kh * KH * 128:(kh + 1) * KH * 128,
                               p * 256:(p + 1) * 256]
                                .rearrange("(kk p) f -> p kk f", p=128))

                    tb_order = range(NTB) if p == 0 else \
                        list(reversed(range(NTB)))
                    for tb in tb_order:
                        t0 = tb * TB
                        cos_tb = cpool1.tile([128, TB], F32, tag="cos")
                        nc.sync.dma_start(cos_tb[:], cosT[:, t0:t0 + TB])
                        sin_tb = cpool1.tile([128, TB], F32, tag="sin")
                        nc.sync.dma_start(sin_tb[:], sinS[:, t0:t0 + TB])
                        if p == 1 and tb == NTB - 1 and last_x[0] is not None:
                            xall = last_x[0]
                        else:
                            xall = xpool.tile([128, KT * TB], F32R, tag="x")
                            for kh in range(2):
                                nc.sync.dma_start(
                                    xall[:, kh * 16 * TB:(kh + 1) * 16 * TB]
                                        .rearrange("p (kk t) -> p kk t", kk=16),
                                    xT[kh * 2048:(kh + 1) * 2048, t0:t0 + TB]
                                        .rearrange("(kk p) t -> p kk t", p=128))
                        if p == 0 and tb == NTB - 1:
                            last_x[0] = xall

                        qs_all = opool.tile([128, 4 * TB], F32R, tag="qs")
                        for m in range(4):
                            ps = pqk.tile([128, TB], F32, tag="qk")
                            for kk in range(KT):
                                nc.tensor.matmul(
                                    ps[:],
                                    wqk_a[:, kk * 512 + m * 128:kk * 512 + (m + 1) * 128],
                                    xall[:, kk * TB:(kk + 1) * TB],
                                    start=(kk == 0), stop=(kk == KT - 1))
                            # RoPE fused into the epilogue: rotate-half via
                            # partition-shifted copies, sign folded into sinS
                            rot = rpool1.tile([128, TB], F32, tag="rot")
                            nc.vector.tensor_copy(rot[0:64, :], ps[64:128, :])
                            nc.vector.tensor_copy(rot[64:128, :], ps[0:64, :])
                            m1_ = rpool1.tile([128, TB], F32, tag="m1")
                            nc.vector.tensor_tensor(m1_[:], ps[:], cos_tb[:],
                                                    op=ALU.mult)
                            m2_ = rpool1.tile([128, TB], F32, tag="m2")
                            nc.vector.tensor_tensor(m2_[:], rot[:], sin_tb[:],
                                                    op=ALU.mult)
                            nc.vector.tensor_tensor(qs_all[:, m * TB:(m + 1) * TB],
                                                    m1_[:], m2_[:], op=ALU.add)
                        # one DMA: rows (p*4+m)*128 for m in 0..4
                        nc.sync.dma_start(
                            qkT_d[p * 512:(p + 1) * 512, t0:t0 + TB]
                                .rearrange("(m pp) t -> pp m t", pp=128),
                            qs_all[:].rearrange("pp (m t) -> pp m t", m=4))

                        vs_all = opool.tile([128, 2 * 256], F32R, tag="vs")
                        for mt in range(2):
                            ps = pv.tile([128, 256], F32, tag="v")
                            for kk in range(KT):
                                nc.tensor.matmul(
                                    ps[:],
                                    xall[:, kk * TB + mt * 128:kk * TB + (mt + 1) * 128],
                                    wv_a[:, kk * 256:(kk + 1) * 256],
                                    start=(kk == 0), stop=(kk == KT - 1))
                            nc.vector.tensor_copy(vs_all[:, mt * 256:(mt + 1) * 256], ps[:])
                        nc.sync.dma_start(
                            v_d[t0:t0 + TB, p * 256:(p + 1) * 256]
                                .rearrange("(mt pp) f -> pp mt f", pp=128),
                            vs_all[:].rearrange("pp (mt f) -> pp mt f", mt=2))

            # ---------------- Phase 2+3: attention + W_o ----------------
            if "p2" in phases:
              with ExitStack() as ctx:
                wopool = ctx.enter_context(tc.tile_pool(name="p2wo", bufs=1))
                mpool = ctx.enter_context(tc.tile_pool(name="p2m", bufs=1))
                m2pool = ctx.enter_context(tc.tile_pool(name="p2m2", bufs=2))
                tpool = ctx.enter_context(tc.tile_pool(name="p2t", bufs=2))
                epool = ctx.enter_context(tc.tile_pool(name="p2e", bufs=3))
                efpool = ctx.enter_context(tc.tile_pool(name="p2ef", bufs=10))
                apool = ctx.enter_context(tc.tile_pool(name="p2a", bufs=2))
                opool = ctx.enter_context(tc.tile_pool(name="p2o", bufs=3))
                ps_s = ctx.enter_context(tc.tile_pool(name="p2ps", bufs=3, space="PSUM"))
                ps_o = ctx.enter_context(tc.tile_pool(name="p2po", bufs=2, space="PSUM"))
                ps_d = ctx.enter_context(tc.tile_pool(name="p2pd", bufs=1, space="PSUM"))
                ps_av = ctx.enter_context(tc.tile_pool(name="p2pav", bufs=2, space="PSUM"))

                # W_o resident: one tile; DMA issued after the first head's
                # input loads so attention starts immediately
                wo_a = wopool.tile([128, HC * H], F32R, tag="wo")

                for b in range(B):
                    bs = b * S
                    mask_state = [None]

                    def load_mask():
                        mask_a = m2pool.tile([128, 4 * S], BF16, tag="maskA")
                        nc.sync.dma_start(
                            mask_a[:].rearrange("p (mt t) -> p mt t", mt=4),
                            maskT[b, 0:512].rearrange("(mt p) t -> p mt t", p=128))
                        mask_bb = m2pool.tile([128, 4 * S], BF16, tag="maskB")
                        nc.sync.dma_start(
                            mask_bb[:].rearrange("p (mt t) -> p mt t", mt=4),
                            maskT[b, 512:1024].rearrange("(mt p) t -> p mt t", p=128))
                        mask_state[0] = (mask_a, mask_bb)

                    if b > 0:
                        load_mask()
                    attn_t = []
                    for l in range(HC):
                        rq = (4 * (l // 2) + 2 * (l % 2)) * 128
                        vcol = (l // 2) * 256 + (l % 2) * 128

                        # rope'd q,k load: [p, j(q/k), t] (1 DMA)
                        kq = tpool.tile([128, 2 * S], F32R, tag="kqraw")
                        nc.sync.dma_start(
                            kq[:].rearrange("p (j t) -> p j t", j=2),
                            qkT_d[rq:rq + 256, bs:bs + S]
                                .rearrange("(j p) t -> p j t", p=128))
                        vt_ = tpool.tile([128, 8 * 128], F32R, tag="vt")
                        nc.sync.dma_start(
                            vt_[:].rearrange("p (kt d) -> p kt d", kt=8),
                            v_d[bs:bs + S, vcol:vcol + 128]
                               .rearrange("(kt p) d -> p kt d", p=128))
                        q_rope = kq[:, 0:S]
                        k_rope = kq[:, S:2 * S]
                        if mask_state[0] is None:
                            load_mask()
                        mask_halves = mask_state[0]
                        if b == 0 and l >= 1:
                            lc = l - 1
                            nc.sync.dma_start(wo_a[:, lc * H:(lc + 1) * H],
                                              wo[lc * 128:(lc + 1) * 128, :])
                            if l == 3:
                                nc.sync.dma_start(wo_a[:, 3 * H:4 * H],
                                                  wo[3 * 128:4 * 128, :])

                        at = apool.tile([128, S], F32R, tag=f"attn{l}")
                        for nt in range(2):
                            nq = nt * 512
                            psd = ps_d.tile([1, 512], F32, tag="dbc")
                            psav = ps_av.tile([128, 512], F32, tag="av")
                            ef_tiles = []
                            for mt in range(8):
                                pss = ps_s.tile([128, 512], F32, tag="s")
                                nc.tensor.matmul(
                                    pss[:], k_rope[:, mt * 128:(mt + 1) * 128],
                                    q_rope[:, nq:nq + 512], start=True, stop=True)
                                ef0 = epool.tile([128, 512], F32, tag="ef0")
                                nc.scalar.activation(ef0[:], pss[:], AF.Exp,
                                                     scale=SCALE)
                                ef = efpool.tile([128, 512], F32R, tag="ef")
                                mh = mask_halves[mt // 4]
                                msl = mh[:, (mt % 4) * S + nq:(mt % 4) * S + nq + 512]
                                eng = nc.gpsimd if mt % 4 == 3 else nc.vector
                                eng.tensor_tensor(ef[:], ef0[:], msl, op=ALU.mult)
                                ef_tiles.append(ef)
                                nc.tensor.matmul(
                                    psav[:], vt_[:, mt * 128:(mt + 1) * 128], ef[:],
                                    start=(mt == 0), stop=(mt == 7))
                            for mt in range(8):
                                nc.tensor.matmul(psd[:], o128[:], ef_tiles[mt][:],
                                                 start=(mt == 0), stop=(mt == 7))
                            rd = epool.tile([1, 512], F32R, tag="rd")
                            nc.vector.reciprocal(rd[:], psd[:])
                            psbc = ps_d.tile([128, 512], F32, tag="dbc")
                            nc.tensor.matmul(psbc[:], o1[:], rd[:], start=True, stop=True)
                            bcs = epool.tile([128, 512], F32, tag="bcs")
                            nc.scalar.copy(bcs[:], psbc[:])
                            nc.vector.tensor_tensor(at[:, nq:nq + 512], psav[:],
                                                    bcs[:], op=ALU.mult)
                        attn_t.append(at)

                    # W_o partial for batch b's tokens (half-row output tiles)
                    for m in range(8):
                        for half in range(4):
                            os_ = opool.tile([128, 1024], F32, tag="os")
                            for n in range(2):
                                nf = half * 1024 + n * 512
                                if b == B - 1 and (2 * half + n) % 2 == 1:
                                    pso = ps_av.tile([128, 512], F32, tag="av")
                                else:
                                    pso = ps_o.tile([128, 512], F32, tag="o")
                                for l in range(HC):
                                    nc.tensor.matmul(
                                        pso[:], attn_t[l][:, m * 128:(m + 1) * 128],
                                        wo_a[:, l * H + nf:l * H + nf + 512],
                                        start=(l == 0), stop=(l == HC - 1))
                                if n % 2 == 0:
                                    nc.vector.tensor_copy(
                                        os_[:, n * 512:(n + 1) * 512], pso[:])
                                else:
                                    nc.scalar.copy(
                                        os_[:, n * 512:(n + 1) * 512], pso[:])
                            nc.sync.dma_start(
                                out_p[bs + m * 128:bs + (m + 1) * 128,
                                      half * 1024:(half + 1) * 1024], os_[:])
    nc.compile()
    return nc


def _host_prep(hidden_states, W_pack, W_o, attention_mask, position_ids):
    import ml_dtypes
    hidden_states = np.asarray(hidden_states, dtype=np.float32)
    W_pack = np.asarray(W_pack, dtype=np.float32)
    W_o = np.asarray(W_o, dtype=np.float32)
    attention_mask = np.asarray(attention_mask, dtype=np.float32)
    pos = np.asarray(position_ids)

    xT = np.ascontiguousarray(hidden_states.reshape(T, H).T)
    # exp(mask): softmax mask applied multiplicatively after exp
    maskT = np.ascontiguousarray(
        np.exp(attention_mask[:, 0].transpose(0, 2, 1)).astype(ml_dtypes.bfloat16))

    inv = (1.0 / (ROPE_BASE ** (np.arange(0, D, 2, dtype=np.float64) / D)))
    inv = np.concatenate([inv, inv])                       # [D]
    ang = pos.astype(np.float64).reshape(T)[None, :] * inv[:, None]   # [D, T]
    cosT = np.cos(ang).astype(np.float32)
    sinT = np.sin(ang).astype(np.float32)
    sinS = sinT.copy()
    sinS[:64] = -sinT[:64]
    cosT = np.ascontiguousarray(cosT)
    sinS = np.ascontiguousarray(sinS)

    in_maps = []
    for c in range(N_CORES):
        h0 = c * HC
        # wqk column order per pass p: [q_{2p}, k_{2p}, q_{2p+1}, k_{2p+1}]
        qcols = [W_pack[:, (h0 + l) * D:(h0 + l + 1) * D] for l in range(HC)]
        kcols = [W_pack[:, H + (h0 + l) * D:H + (h0 + l + 1) * D] for l in range(HC)]
        vcols = [W_pack[:, 2 * H + (h0 + l) * D:2 * H + (h0 + l + 1) * D]
                 for l in range(HC)]
        wqk_np = np.ascontiguousarray(np.concatenate(
            [qcols[0], kcols[0], qcols[1], kcols[1],
             qcols[2], kcols[2], qcols[3], kcols[3]], axis=1))
        wv_np = np.ascontiguousarray(np.concatenate(vcols, axis=1))
        wo_np = np.ascontiguousarray(W_o[h0 * D:(h0 + HC) * D, :])
        in_maps.append({
            "xT": xT, "wqk": wqk_np, "wv": wv_np, "wo": wo_np,
            "cosT": cosT, "sinS": sinS, "maskT": maskT,
        })
    return in_maps


def kernel(hidden_states, W_pack, W_o, attention_mask, position_ids):
    if "nc" not in _CACHE:
        _CACHE["nc"] = _build_module()
    nc = _CACHE["nc"]
    in_maps = _host_prep(hidden_states, W_pack, W_o, attention_mask, position_ids)
    res = bass_utils.run_bass_kernel_spmd(nc, in_maps, core_ids=list(range(N_CORES)))
    out = res.results[0]["out_p"].astype(np.float64)
    for c in range(1, N_CORES):
        out += res.results[c]["out_p"]
    return out.reshape(B, S, H).astype(np.float32)


# revision 50
# speedup vs baseline: 1.3119x; 1.0014x over previous
"""Trainium2 Bass kernel for nn_Attention_60567628808865.

Dense transformer attention block (B=4, S=1024, H=4096, NH=32, D=128):
  qkv = x @ W_pack; RoPE(q, k); causal-masked softmax attention; out @ W_o.

Sharding: tensor-parallel over heads across 8 NeuronCores. Each core computes
4 heads end-to-end (QKV projection with its W_pack column slice, attention,
and its W_o row-slice partial of the output projection); the host sums the 8
partial outputs.

All matmuls run in float32r (TF32) at full PE rate; accumulation is fp32 in
PSUM. Everything on-chip works in a transposed layout (features on the
partition axis) so no transposes are needed anywhere:
  qT/kT [d, t] <- lhsT=W_qk, rhs=xT      scoresT [tk, tq] <- lhsT=kT, rhs=qT
  v [t, d]     <- lhsT=xT,   rhs=W_v     attnT [d, tq]    <- lhsT=v,  rhs=expT
  out [t, f]   <- lhsT=attnT, rhs=W_o
Softmax runs unnormalized (no max-subtraction; scores are O(1) by
construction and exp(-1e9)=0), with the denominator computed by a ones-vector
matmul accumulated in PSUM and applied after PV via a K=1 broadcast matmul.
RoPE's rotate-half is a partition shift, done for free in the DMA that loads
q/k back from scratch, with the sign folded into the host-built sin table.
DMAs are batched into multi-dim-AP transfers (the HWDGE issue path costs
~625ns per DMA instruction, so many small DMAs throttle the PE).
"""
import numpy as np

import concourse.bass as bass  # noqa: F401  (AP types come via tile/bacc)
import concourse.tile as tile
from contextlib import ExitStack
from concourse import bacc, mybir
from concourse import bass_utils

F32 = mybir.dt.float32
F32R = mybir.dt.float32r
BF16 = mybir.dt.bfloat16
AF = mybir.ActivationFunctionType
ALU = mybir.AluOpType

B, S, H, NH = 4, 1024, 4096, 32
D = H // NH          # 128
T = B * S            # 4096 tokens
N_CORES = 8
HC = NH // N_CORES   # 4 heads per core
SCALE = float(1.0 / np.sqrt(D))
ROPE_BASE = 10000.0

TB = 256             # phase-1 token block (moving dim; >=256 keeps f32r at full rate)
NTB = T // TB        # 16
KT = H // 128        # 32 contraction tiles

_CACHE = {}


def _build_module(phases=("p1", "p2")):
    nc = bacc.Bacc("TRN2", target_bir_lowering=False, debug=False,
                   num_devices=N_CORES)

    xT = nc.dram_tensor("xT", [H, T], F32R, kind="ExternalInput").ap()
    wqk = nc.dram_tensor("wqk", [H, 2 * HC * D], F32R, kind="ExternalInput").ap()
    wv = nc.dram_tensor("wv", [H, HC * D], F32R, kind="ExternalInput").ap()
    wo = nc.dram_tensor("wo", [HC * D, H], F32R, kind="ExternalInput").ap()
    cosT = nc.dram_tensor("cosT", [D, T], F32, kind="ExternalInput").ap()
    sinS = nc.dram_tensor("sinS", [D, T], F32, kind="ExternalInput").ap()
    maskT = nc.dram_tensor("maskT", [B, S, S], BF16, kind="ExternalInput").ap()
    out_p = nc.dram_tensor("out_p", [T, H], F32, kind="ExternalOutput").ap()
    ones128 = nc.inline_tensor(np.ones((128, 1), np.float32), "ones128").ap().bitcast(F32R)
    ones1 = nc.inline_tensor(np.ones((1, 128), np.float32), "ones1").ap().bitcast(F32R)

    with tile.TileContext(nc) as tc, \
         nc.allow_low_precision(reason="tf32 matmuls; verified against reference"):
        with ExitStack() as octx:
            dram = octx.enter_context(tc.tile_pool(name="dram", bufs=1, space="DRAM"))
            cpool = octx.enter_context(tc.tile_pool(name="consts", bufs=1))
            # scratch: qkT rows (pass p, m): [q_2p, k_2p, q_2p+1, k_2p+1]
            qkT_d = dram.tile([2 * HC * D, T], F32R)
            v_d = dram.tile([T, HC * D], F32R)

            o128 = cpool.tile([128, 1], F32R)
            nc.sync.dma_start(o128[:], ones128[:])
            o1 = cpool.tile([1, 128], F32R)
            nc.sync.dma_start(o1[:], ones1[:])

            # ---------------- Phase 1: QKV projection ----------------
            if "p1" in phases:
              with ExitStack() as ctx:
                wpool = ctx.enter_context(tc.tile_pool(name="p1w", bufs=1))
                xpool = ctx.enter_context(tc.tile_pool(name="p1x", bufs=2))
                opool = ctx.enter_context(tc.tile_pool(name="p1o", bufs=2))
                cpool1 = ctx.enter_context(tc.tile_pool(name="p1cs", bufs=2))
                rpool1 = ctx.enter_context(tc.tile_pool(name="p1rope", bufs=2))
                pqk = ctx.enter_context(tc.tile_pool(name="p1pqk", bufs=6, space="PSUM"))
                pv = ctx.enter_context(tc.tile_pool(name="p1pv", bufs=2, space="PSUM"))

                last_x = [None]

                def load_tb_inputs(p, tb):
                    t0 = tb * TB
                    cos_tb = cpool1.tile([128, TB], F32, tag="cos")
                    nc.sync.dma_start(cos_tb[:], cosT[:, t0:t0 + TB])
                    sin_tb = cpool1.tile([128, TB], F32, tag="sin")
                    nc.sync.dma_start(sin_tb[:], sinS[:, t0:t0 + TB])
                    if p == 1 and tb == NTB - 1 and last_x[0] is not None:
                        xall = last_x[0]
                    else:
                        xall = xpool.tile([128, KT * TB], F32R, tag="x")
                        for kh in range(2):
                            nc.sync.dma_start(
                                xall[:, kh * 16 * TB:(kh + 1) * 16 * TB]
                                    .rearrange("p (kk t) -> p kk t", kk=16),
                                xT[kh * 2048:(kh + 1) * 2048, t0:t0 + TB]
                                    .rearrange("(kk p) t -> p kk t", p=128))
                    if p == 0 and tb == NTB - 1:
                        last_x[0] = xall
                    return cos_tb, sin_tb, xall

                for p in range(2):
                    # resident weights, split into half-K DMAs so the first
                    # accumulation chains can start at half-load
                    first_inputs = load_tb_inputs(p, 0 if p == 0 else NTB - 1)
                    # weights as independent half-K tiles: pass p+1's low half
                    # can reload while pass p still reads the high half
                    KH = KT // 2
                    wqk_lo = wpool.tile([128, KH * 512], F32R, tag="wqk_lo")
                    wqk_hi = wpool.tile([128, KH * 512], F32R, tag="wqk_hi")
                    wv_lo = wpool.tile([128, KH * 256], F32R, tag="wv_lo")
                    wv_hi = wpool.tile([128, KH * 256], F32R, tag="wv_hi")
                    for kh, wt in ((0, wqk_lo), (1, wqk_hi)):
                        nc.sync.dma_start(
                            wt[:].rearrange("p (kk f) -> p kk f", kk=KH),
                            wqk[kh * KH * 128:(kh + 1) * KH * 128,
                                p * 512:(p + 1) * 512]
                                .rearrange("(kk p) f -> p kk f", p=128))
                    for kh, wt in ((0, wv_lo), (1, wv_hi)):
                        nc.sync.dma_start(
                            wt[:].rearrange("p (kk f) -> p kk f", kk=KH),
                            wv[kh * KH * 128:(kh + 1) * KH * 128,
                               p * 256:(p + 1) * 256]
                                .rearrange("(kk p) f -> p kk f", p=128))

                    def wqk_sl(kk, c0, c1):
                        wt = wqk_lo if kk < KH else wqk_hi
                        return wt[:, (kk % KH) * 512 + c0:(kk % KH) * 512 + c1]

                    def wv_sl(kk):
                        wt = wv_lo if kk < KH else wv_hi
                        return wt[:, (kk % KH) * 256:(kk % KH + 1) * 256]

                    tb_order = list(range(NTB)) if p == 0 else \
                        list(reversed(range(NTB)))
                    for tb in tb_order:
                        t0 = tb * TB
                        if tb == tb_order[0]:
                            cos_tb, sin_tb, xall = first_inputs
                        else:
                            cos_tb, sin_tb, xall = load_tb_inputs(p, tb)

                        qs_all = opool.tile([128, 4 * TB], F32R, tag="qs")
                        for m in range(4):
                            ps = pqk.tile([128, TB], F32, tag="qk")
                            for kk in range(KT):
                                nc.tensor.matmul(
                                    ps[:],
                                    wqk_sl(kk, m * 128, (m + 1) * 128),
                                    xall[:, kk * TB:(kk + 1) * TB],
                                    start=(kk == 0), stop=(kk == KT - 1))
                            # RoPE fused into the epilogue: rotate-half via
                            # partition-shifted copies, sign folded into sinS
                            rot = rpool1.tile([128, TB], F32, tag="rot")
                            nc.vector.tensor_copy(rot[0:64, :], ps[64:128, :])
                            nc.vector.tensor_copy(rot[64:128, :], ps[0:64, :])
                            m1_ = rpool1.tile([128, TB], F32, tag="m1")
                            nc.vector.tensor_tensor(m1_[:], ps[:], cos_tb[:],
                                                    op=ALU.mult)
                            m2_ = rpool1.tile([128, TB], F32, tag="m2")
                            nc.vector.tensor_tensor(m2_[:], rot[:], sin_tb[:],
                                                    op=ALU.mult)
                            nc.vector.tensor_tensor(qs_all[:, m * TB:(m + 1) * TB],
                                                    m1_[:], m2_[:], op=ALU.add)
                        # one DMA: rows (p*4+m)*128 for m in 0..4
                        nc.sync.dma_start(
                            qkT_d[p * 512:(p + 1) * 512, t0:t0 + TB]
                                .rearrange("(m pp) t -> pp m t", pp=128),
                            qs_all[:].rearrange("pp (m t) -> pp m t", m=4))

                        vs_all = opool.tile([128, 2 * 256], F32R, tag="vs")
                        for mt in range(2):
                            ps = pv.tile([128, 256], F32, tag="v")
                            for kk in range(KT):
                                nc.tensor.matmul(
                                    ps[:],
                                    xall[:, kk * TB + mt * 128:kk * TB + (mt + 1) * 128],
                                    wv_sl(kk),
                                    start=(kk == 0), stop=(kk == KT - 1))
                            nc.vector.tensor_copy(vs_all[:, mt * 256:(mt + 1) * 256], ps[:])
                        nc.sync.dma_start(
                            v_d[t0:t0 + TB, p * 256:(p + 1) * 256]
                                .rearrange("(mt pp) f -> pp mt f", pp=128),
                            vs_all[:].rearrange("pp (mt f) -> pp mt f", mt=2))

            # ---------------- Phase 2+3: attention + W_o ----------------
            if "p2" in phases:
              with ExitStack() as ctx:
                wopool = ctx.enter_context(tc.tile_pool(name="p2wo", bufs=1))
                mpool = ctx.enter_context(tc.tile_pool(name="p2m", bufs=1))
                m2pool = ctx.enter_context(tc.tile_pool(name="p2m2", bufs=2))
                tpool = ctx.enter_context(tc.tile_pool(name="p2t", bufs=2))
                epool = ctx.enter_context(tc.tile_pool(name="p2e", bufs=3))
                efpool = ctx.enter_context(tc.tile_pool(name="p2ef", bufs=10))
                apool = ctx.enter_context(tc.tile_pool(name="p2a", bufs=2))
                opool = ctx.enter_context(tc.tile_pool(name="p2o", bufs=3))
                ps_s = ctx.enter_context(tc.tile_pool(name="p2ps", bufs=3, space="PSUM"))
                ps_o = ctx.enter_context(tc.tile_pool(name="p2po", bufs=2, space="PSUM"))
                ps_d = ctx.enter_context(tc.tile_pool(name="p2pd", bufs=1, space="PSUM"))
                ps_av = ctx.enter_context(tc.tile_pool(name="p2pav", bufs=2, space="PSUM"))

                # W_o resident: one tile; DMA issued after the first head's
                # input loads so attention starts immediately
                wo_a = wopool.tile([128, HC * H], F32R, tag="wo")

                for b in range(B):
                    bs = b * S
                    mask_state = [None]

                    def load_mask():
                        mask_a = m2pool.tile([128, 4 * S], BF16, tag="maskA")
                        nc.sync.dma_start(
                            mask_a[:].rearrange("p (mt t) -> p mt t", mt=4),
                            maskT[b, 0:512].rearrange("(mt p) t -> p mt t", p=128))
                        mask_bb = m2pool.tile([128, 4 * S], BF16, tag="maskB")
                        nc.sync.dma_start(
                            mask_bb[:].rearrange("p (mt t) -> p mt t", mt=4),
                            maskT[b, 512:1024].rearrange("(mt p) t -> p mt t", p=128))
                        mask_state[0] = (mask_a, mask_bb)

                    if b > 0:
                        load_mask()
                    attn_t = []
                    for l in range(HC):
                        rq = (4 * (l // 2) + 2 * (l % 2)) * 128
                        vcol = (l // 2) * 256 + (l % 2) * 128

                        # rope'd q,k load: [p, j(q/k), t] (1 DMA)
                        kq = tpool.tile([128, 2 * S], F32R, tag="kqraw")
                        nc.sync.dma_start(
                            kq[:].rearrange("p (j t) -> p j t", j=2),
                            qkT_d[rq:rq + 256, bs:bs + S]
                                .rearrange("(j p) t -> p j t", p=128))
                        vt_ = tpool.tile([128, 8 * 128], F32R, tag="vt")
                        nc.sync.dma_start(
                            vt_[:].rearrange("p (kt d) -> p kt d", kt=8),
                            v_d[bs:bs + S, vcol:vcol + 128]
                               .rearrange("(kt p) d -> p kt d", p=128))
                        q_rope = kq[:, 0:S]
                        k_rope = kq[:, S:2 * S]
                        if mask_state[0] is None:
                            load_mask()
                        mask_halves = mask_state[0]
                        if b == 0 and l >= 1:
                            lc = l - 1
                            nc.sync.dma_start(wo_a[:, lc * H:(lc + 1) * H],
                                              wo[lc * 128:(lc + 1) * 128, :])
                            if l == 3:
                                nc.sync.dma_start(wo_a[:, 3 * H:4 * H],
                                                  wo[3 * 128:4 * 128, :])

                        at = apool.tile([128, S], F32R, tag=f"attn{l}")
                        for nt in range(2):
                            nq = nt * 512
                            psd = ps_d.tile([1, 512], F32, tag="dbc")
                            psav = ps_av.tile([128, 512], F32, tag="av")
                            ef_tiles = []
                            for mt in range(8):
                                pss = ps_s.tile([128, 512], F32, tag="s")
                                nc.tensor.matmul(
                                    pss[:], k_rope[:, mt * 128:(mt + 1) * 128],
                                    q_rope[:, nq:nq + 512], start=True, stop=True)
                                ef0 = epool.tile([128, 512], F32, tag="ef0")
                                nc.scalar.activation(ef0[:], pss[:], AF.Exp,
                                                     scale=SCALE)
                                ef = efpool.tile([128, 512], F32R, tag="ef")
                                mh = mask_halves[mt // 4]
                                msl = mh[:, (mt % 4) * S + nq:(mt % 4) * S + nq + 512]
                                eng = nc.gpsimd if mt % 4 == 3 else nc.vector
                                eng.tensor_tensor(ef[:], ef0[:], msl, op=ALU.mult)
                                ef_tiles.append(ef)
                                nc.tensor.matmul(
                                    psav[:], vt_[:, mt * 128:(mt + 1) * 128], ef[:],
                                    start=(mt == 0), stop=(mt == 7))
                            for mt in range(8):
                                nc.tensor.matmul(psd[:], o128[:], ef_tiles[mt][:],
                                                 start=(mt == 0), stop=(mt == 7))
                            rd = epool.tile([1, 512], F32R, tag="rd")
                            nc.vector.reciprocal(rd[:], psd[:])
                            psbc = ps_d.tile([128, 512], F32, tag="dbc")
                            nc.tensor.matmul(psbc[:], o1[:], rd[:], start=True, stop=True)
                            bcs = epool.tile([128, 512], F32, tag="bcs")
                            nc.scalar.copy(bcs[:], psbc[:])
                            nc.vector.tensor_tensor(at[:, nq:nq + 512], psav[:],
                                                    bcs[:], op=ALU.mult)
                        attn_t.append(at)

                    # W_o partial for batch b's tokens (half-row output tiles)
                    for m in range(8):
                        for half in range(4):
                            os_ = opool.tile([128, 1024], F32, tag="os")
                            for n in range(2):
                                nf = half * 1024 + n * 512
                                if b == B - 1 and (2 * half + n) % 2 == 1:
                                    pso = ps_av.tile([128, 512], F32, tag="av")
                                else:
                                    pso = ps_o.tile([128, 512], F32, tag="o")
                                for l in range(HC):
                                    nc.tensor.matmul(
                                        pso[:], attn_t[l][:, m * 128:(m + 1) * 128],
                                        wo_a[:, l * H + nf:l * H + nf + 512],
                                        start=(l == 0), stop=(l == HC - 1))
                                if n % 2 == 0:
                                    nc.vector.tensor_copy(
                                        os_[:, n * 512:(n + 1) * 512], pso[:])
                                else:
                                    nc.scalar.copy(
                                        os_[:, n * 512:(n + 1) * 512], pso[:])
                            nc.sync.dma_start(
                                out_p[bs + m * 128:bs + (m + 1) * 128,
                                      half * 1024:(half + 1) * 1024], os_[:])
    nc.compile()
    return nc


def _host_prep(hidden_states, W_pack, W_o, attention_mask, position_ids):
    import ml_dtypes
    hidden_states = np.asarray(hidden_states, dtype=np.float32)
    W_pack = np.asarray(W_pack, dtype=np.float32)
    W_o = np.asarray(W_o, dtype=np.float32)
    attention_mask = np.asarray(attention_mask, dtype=np.float32)
    pos = np.asarray(position_ids)

    xT = np.ascontiguousarray(hidden_states.reshape(T, H).T)
    # exp(mask): softmax mask applied multiplicatively after exp
    maskT = np.ascontiguousarray(
        np.exp(attention_mask[:, 0].transpose(0, 2, 1)).astype(ml_dtypes.bfloat16))

    inv = (1.0 / (ROPE_BASE ** (np.arange(0, D, 2, dtype=np.float64) / D)))
    inv = np.concatenate([inv, inv])                       # [D]
    ang = pos.astype(np.float64).reshape(T)[None, :] * inv[:, None]   # [D, T]
    cosT = np.cos(ang).astype(np.float32)
    sinT = np.sin(ang).astype(np.float32)
    sinS = sinT.copy()
    sinS[:64] = -sinT[:64]
    cosT = np.ascontiguousarray(cosT)
    sinS = np.ascontiguousarray(sinS)

    in_maps = []
    for c in range(N_CORES):
        h0 = c * HC
        # wqk column order per pass p: [q_{2p}, k_{2p}, q_{2p+1}, k_{2p+1}]
        qcols = [W_pack[:, (h0 + l) * D:(h0 + l + 1) * D] for l in range(HC)]
        kcols = [W_pack[:, H + (h0 + l) * D:H + (h0 + l + 1) * D] for l in range(HC)]
        vcols = [W_pack[:, 2 * H + (h0 + l) * D:2 * H + (h0 + l + 1) * D]
                 for l in range(HC)]
        wqk_np = np.ascontiguousarray(np.concatenate(
            [qcols[0], kcols[0], qcols[1], kcols[1],
             qcols[2], kcols[2], qcols[3], kcols[3]], axis=1))
        wv_np = np.ascontiguousarray(np.concatenate(vcols, axis=1))
        wo_np = np.ascontiguousarray(W_o[h0 * D:(h0 + HC) * D, :])
        in_maps.append({
            "xT": xT, "wqk": wqk_np, "wv": wv_np, "wo": wo_np,
            "cosT": cosT, "sinS": sinS, "maskT": maskT,
        })
    return in_maps


def kernel(hidden_states, W_pack, W_o, attention_mask, position_ids):
    if "nc" not in _CACHE:
        _CACHE["nc"] = _build_module()
    nc = _CACHE["nc"]
    in_maps = _host_prep(hidden_states, W_pack, W_o, attention_mask, position_ids)
    res = bass_utils.run_bass_kernel_spmd(nc, in_maps, core_ids=list(range(N_CORES)))
    out = res.results[0]["out_p"].astype(np.float64)
    for c in range(1, N_CORES):
        out += res.results[c]["out_p"]
    return out.reshape(B, S, H).astype(np.float32)


# revision 53
# speedup vs baseline: 1.3137x; 1.0013x over previous
"""Trainium2 Bass kernel for nn_Attention_60567628808865.

Dense transformer attention block (B=4, S=1024, H=4096, NH=32, D=128):
  qkv = x @ W_pack; RoPE(q, k); causal-masked softmax attention; out @ W_o.

Sharding: tensor-parallel over heads across 8 NeuronCores. Each core computes
4 heads end-to-end (QKV projection with its W_pack column slice, attention,
and its W_o row-slice partial of the output projection); the host sums the 8
partial outputs.

All matmuls run in float32r (TF32) at full PE rate; accumulation is fp32 in
PSUM. Everything on-chip works in a transposed layout (features on the
partition axis) so no transposes are needed anywhere:
  qT/kT [d, t] <- lhsT=W_qk, rhs=xT      scoresT [tk, tq] <- lhsT=kT, rhs=qT
  v [t, d]     <- lhsT=xT,   rhs=W_v     attnT [d, tq]    <- lhsT=v,  rhs=expT
  out [t, f]   <- lhsT=attnT, rhs=W_o
Softmax runs unnormalized (no max-subtraction; scores are O(1) by
construction and exp(-1e9)=0), with the denominator computed by a ones-vector
matmul accumulated in PSUM and applied after PV via a K=1 broadcast matmul.
RoPE's rotate-half is a partition shift, done for free in the DMA that loads
q/k back from scratch, with the sign folded into the host-built sin table.
DMAs are batched into multi-dim-AP transfers (the HWDGE issue path costs
~625ns per DMA instruction, so many small DMAs throttle the PE).
"""
import numpy as np

import concourse.bass as bass  # noqa: F401  (AP types come via tile/bacc)
import concourse.tile as tile
from contextlib import ExitStack
from concourse import bacc, mybir
from concourse import bass_utils

F32 = mybir.dt.float32
F32R = mybir.dt.float32r
BF16 = mybir.dt.bfloat16
AF = mybir.ActivationFunctionType
ALU = mybir.AluOpType

B, S, H, NH = 4, 1024, 4096, 32
D = H // NH          # 128
T = B * S            # 4096 tokens
N_CORES = 8
HC = NH // N_CORES   # 4 heads per core
SCALE = float(1.0 / np.sqrt(D))
ROPE_BASE = 10000.0

TB = 256             # phase-1 token block (moving dim; >=256 keeps f32r at full rate)
NTB = T // TB        # 16
KT = H // 128        # 32 contraction tiles

_CACHE = {}


def _build_module(phases=("p1", "p2")):
    nc = bacc.Bacc("TRN2", target_bir_lowering=False, debug=False,
                   num_devices=N_CORES)

    xT = nc.dram_tensor("xT", [H, T], F32R, kind="ExternalInput").ap()
    wqk = nc.dram_tensor("wqk", [H, 2 * HC * D], F32R, kind="ExternalInput").ap()
    wv = nc.dram_tensor("wv", [H, HC * D], F32R, kind="ExternalInput").ap()
    wo = nc.dram_tensor("wo", [HC * D, H], F32R, kind="ExternalInput").ap()
    cosT = nc.dram_tensor("cosT", [D, T], F32, kind="ExternalInput").ap()
    sinS = nc.dram_tensor("sinS", [D, T], F32, kind="ExternalInput").ap()
    maskT = nc.dram_tensor("maskT", [B, S, S], BF16, kind="ExternalInput").ap()
    out_p = nc.dram_tensor("out_p", [T, H], F32, kind="ExternalOutput").ap()
    ones128 = nc.inline_tensor(np.ones((128, 1), np.float32), "ones128").ap().bitcast(F32R)
    ones1 = nc.inline_tensor(np.ones((1, 128), np.float32), "ones1").ap().bitcast(F32R)

    with tile.TileContext(nc) as tc, \
         nc.allow_low_precision(reason="tf32 matmuls; verified against reference"):
        with ExitStack() as octx:
            dram = octx.enter_context(tc.tile_pool(name="dram", bufs=1, space="DRAM"))
            cpool = octx.enter_context(tc.tile_pool(name="consts", bufs=1))
            # scratch: qkT rows (pass p, m): [q_2p, k_2p, q_2p+1, k_2p+1]
            qkT_d = dram.tile([2 * HC * D, T], F32R)
            v_d = dram.tile([T, HC * D], F32R)

            o128 = cpool.tile([128, 1], F32R)
            nc.sync.dma_start(o128[:], ones128[:])
            o1 = cpool.tile([1, 128], F32R)
            nc.sync.dma_start(o1[:], ones1[:])

            # ---------------- Phase 1: QKV projection ----------------
            if "p1" in phases:
              with ExitStack() as ctx:
                wpool = ctx.enter_context(tc.tile_pool(name="p1w", bufs=1))
                xpool = ctx.enter_context(tc.tile_pool(name="p1x", bufs=2))
                opool = ctx.enter_context(tc.tile_pool(name="p1o", bufs=2))
                cpool1 = ctx.enter_context(tc.tile_pool(name="p1cs", bufs=2))
                rpool1 = ctx.enter_context(tc.tile_pool(name="p1rope", bufs=2))
                pqk = ctx.enter_context(tc.tile_pool(name="p1pqk", bufs=6, space="PSUM"))
                pv = ctx.enter_context(tc.tile_pool(name="p1pv", bufs=2, space="PSUM"))

                last_x = [None]

                def load_tb_inputs(p, tb):
                    t0 = tb * TB
                    cos_tb = cpool1.tile([128, TB], F32, tag="cos")
                    nc.sync.dma_start(cos_tb[:], cosT[:, t0:t0 + TB])
                    sin_tb = cpool1.tile([128, TB], F32, tag="sin")
                    nc.sync.dma_start(sin_tb[:], sinS[:, t0:t0 + TB])
                    if p == 1 and tb == NTB - 1 and last_x[0] is not None:
                        xall = last_x[0]
                    else:
                        xall = xpool.tile([128, KT * TB], F32R, tag="x")
                        for kh in range(2):
                            nc.sync.dma_start(
                                xall[:, kh * 16 * TB:(kh + 1) * 16 * TB]
                                    .rearrange("p (kk t) -> p kk t", kk=16),
                                xT[kh * 2048:(kh + 1) * 2048, t0:t0 + TB]
                                    .rearrange("(kk p) t -> p kk t", p=128))
                    if p == 0 and tb == NTB - 1:
                        last_x[0] = xall
                    return cos_tb, sin_tb, xall

                for p in range(2):
                    # resident weights, split into half-K DMAs so the first
                    # accumulation chains can start at half-load
                    first_inputs = load_tb_inputs(p, 0 if p == 0 else NTB - 1)
                    # weights as independent half-K tiles: pass p+1's low half
                    # can reload while pass p still reads the high half
                    KH = KT // 2
                    wqk_lo = wpool.tile([128, KH * 512], F32R, tag="wqk_lo")
                    wqk_hi = wpool.tile([128, KH * 512], F32R, tag="wqk_hi")
                    wv_lo = wpool.tile([128, KH * 256], F32R, tag="wv_lo")
                    wv_hi = wpool.tile([128, KH * 256], F32R, tag="wv_hi")
                    for kh, wt in ((0, wqk_lo), (1, wqk_hi)):
                        nc.sync.dma_start(
                            wt[:].rearrange("p (kk f) -> p kk f", kk=KH),
                            wqk[kh * KH * 128:(kh + 1) * KH * 128,
                                p * 512:(p + 1) * 512]
                                .rearrange("(kk p) f -> p kk f", p=128))
                    for kh, wt in ((0, wv_lo), (1, wv_hi)):
                        nc.sync.dma_start(
                            wt[:].rearrange("p (kk f) -> p kk f", kk=KH),
                            wv[kh * KH * 128:(kh + 1) * KH * 128,
                               p * 256:(p + 1) * 256]
                                .rearrange("(kk p) f -> p kk f", p=128))

                    def wqk_sl(kk, c0, c1):
                        wt = wqk_lo if kk < KH else wqk_hi
                        return wt[:, (kk % KH) * 512 + c0:(kk % KH) * 512 + c1]

                    def wv_sl(kk):
                        wt = wv_lo if kk < KH else wv_hi
                        return wt[:, (kk % KH) * 256:(kk % KH + 1) * 256]

                    tb_order = list(range(NTB)) if p == 0 else \
                        list(reversed(range(NTB)))
                    for tb in tb_order:
                        t0 = tb * TB
                        if tb == tb_order[0]:
                            cos_tb, sin_tb, xall = first_inputs
                        else:
                            cos_tb, sin_tb, xall = load_tb_inputs(p, tb)

                        qs_all = opool.tile([128, 4 * TB], F32R, tag="qs")
                        for m in range(4):
                            ps = pqk.tile([128, TB], F32, tag="qk")
                            for kk in range(KT):
                                nc.tensor.matmul(
                                    ps[:],
                                    wqk_sl(kk, m * 128, (m + 1) * 128),
                                    xall[:, kk * TB:(kk + 1) * TB],
                                    start=(kk == 0), stop=(kk == KT - 1))
                            # RoPE fused into the epilogue: rotate-half via
                            # partition-shifted copies, sign folded into sinS
                            rot = rpool1.tile([128, TB], F32, tag="rot")
                            nc.vector.tensor_copy(rot[0:64, :], ps[64:128, :])
                            nc.vector.tensor_copy(rot[64:128, :], ps[0:64, :])
                            m1_ = rpool1.tile([128, TB], F32, tag="m1")
                            nc.vector.tensor_tensor(m1_[:], ps[:], cos_tb[:],
                                                    op=ALU.mult)
                            m2_ = rpool1.tile([128, TB], F32, tag="m2")
                            nc.vector.tensor_tensor(m2_[:], rot[:], sin_tb[:],
                                                    op=ALU.mult)
                            nc.vector.tensor_tensor(qs_all[:, m * TB:(m + 1) * TB],
                                                    m1_[:], m2_[:], op=ALU.add)
                        # one DMA: rows (p*4+m)*128 for m in 0..4
                        nc.sync.dma_start(
                            qkT_d[p * 512:(p + 1) * 512, t0:t0 + TB]
                                .rearrange("(m pp) t -> pp m t", pp=128),
                            qs_all[:].rearrange("pp (m t) -> pp m t", m=4))

                        vs_all = opool.tile([128, 2 * 256], F32R, tag="vs")
                        for mt in range(2):
                            ps = pv.tile([128, 256], F32, tag="v")
                            for kk in range(KT):
                                nc.tensor.matmul(
                                    ps[:],
                                    xall[:, kk * TB + mt * 128:kk * TB + (mt + 1) * 128],
                                    wv_sl(kk),
                                    start=(kk == 0), stop=(kk == KT - 1))
                            nc.vector.tensor_copy(vs_all[:, mt * 256:(mt + 1) * 256], ps[:])
                        nc.sync.dma_start(
                            v_d[t0:t0 + TB, p * 256:(p + 1) * 256]
                                .rearrange("(mt pp) f -> pp mt f", pp=128),
                            vs_all[:].rearrange("pp (mt f) -> pp mt f", mt=2))

            # ---------------- Phase 2+3: attention + W_o ----------------
            if "p2" in phases:
              with ExitStack() as ctx:
                wopool = ctx.enter_context(tc.tile_pool(name="p2wo", bufs=1))
                mpool = ctx.enter_context(tc.tile_pool(name="p2m", bufs=1))
                m2pool = ctx.enter_context(tc.tile_pool(name="p2m2", bufs=2))
                tpool = ctx.enter_context(tc.tile_pool(name="p2t", bufs=2))
                epool = ctx.enter_context(tc.tile_pool(name="p2e", bufs=3))
                efpool = ctx.enter_context(tc.tile_pool(name="p2ef", bufs=10))
                apool = ctx.enter_context(tc.tile_pool(name="p2a", bufs=2))
                opool = ctx.enter_context(tc.tile_pool(name="p2o", bufs=3))
                ps_s = ctx.enter_context(tc.tile_pool(name="p2ps", bufs=3, space="PSUM"))
                ps_o = ctx.enter_context(tc.tile_pool(name="p2po", bufs=2, space="PSUM"))
                ps_d = ctx.enter_context(tc.tile_pool(name="p2pd", bufs=1, space="PSUM"))
                ps_av = ctx.enter_context(tc.tile_pool(name="p2pav", bufs=2, space="PSUM"))

                # W_o resident: one tile; DMA issued after the first head's
                # input loads so attention starts immediately
                wo_a = wopool.tile([128, HC * H], F32R, tag="wo")

                for b in range(B):
                    bs = b * S
                    mask_state = [None]

                    def load_mask():
                        mask_a = m2pool.tile([128, 4 * S], BF16, tag="maskA")
                        nc.sync.dma_start(
                            mask_a[:].rearrange("p (mt t) -> p mt t", mt=4),
                            maskT[b, 0:512].rearrange("(mt p) t -> p mt t", p=128))
                        mask_bb = m2pool.tile([128, 4 * S], BF16, tag="maskB")
                        nc.sync.dma_start(
                            mask_bb[:].rearrange("p (mt t) -> p mt t", mt=4),
                            maskT[b, 512:1024].rearrange("(mt p) t -> p mt t", p=128))
                        mask_state[0] = (mask_a, mask_bb)

                    if b > 0:
                        load_mask()
                    attn_t = []
                    for l in range(HC):
                        rq = (4 * (l // 2) + 2 * (l % 2)) * 128
                        vcol = (l // 2) * 256 + (l % 2) * 128

                        # rope'd q,k load: [p, j(q/k), t] (1 DMA)
                        kq = tpool.tile([128, 2 * S], F32R, tag="kqraw")
                        nc.sync.dma_start(
                            kq[:].rearrange("p (j t) -> p j t", j=2),
                            qkT_d[rq:rq + 256, bs:bs + S]
                                .rearrange("(j p) t -> p j t", p=128))
                        vt_ = tpool.tile([128, 8 * 128], F32R, tag="vt")
                        nc.sync.dma_start(
                            vt_[:].rearrange("p (kt d) -> p kt d", kt=8),
                            v_d[bs:bs + S, vcol:vcol + 128]
                               .rearrange("(kt p) d -> p kt d", p=128))
                        q_rope = kq[:, 0:S]
                        k_rope = kq[:, S:2 * S]
                        if mask_state[0] is None:
                            load_mask()
                        mask_halves = mask_state[0]
                        if b == 0 and l >= 1:
                            lc = l - 1
                            nc.sync.dma_start(wo_a[:, lc * H:(lc + 1) * H],
                                              wo[lc * 128:(lc + 1) * 128, :])
                            if l == 3:
                                nc.sync.dma_start(wo_a[:, 3 * H:4 * H],
                                                  wo[3 * 128:4 * 128, :])

                        at = apool.tile([128, S], F32R, tag=f"attn{l}")
                        for nt in range(2):
                            nq = nt * 512
                            psd = ps_d.tile([1, 512], F32, tag="dbc")
                            psav = ps_av.tile([128, 512], F32, tag="av")
                            ef_tiles = []
                            for mt in range(8):
                                pss = ps_s.tile([128, 512], F32, tag="s")
                                nc.tensor.matmul(
                                    pss[:], k_rope[:, mt * 128:(mt + 1) * 128],
                                    q_rope[:, nq:nq + 512], start=True, stop=True)
                                ef0 = epool.tile([128, 512], F32, tag="ef0")
                                nc.scalar.activation(ef0[:], pss[:], AF.Exp,
                                                     scale=SCALE)
                                ef = efpool.tile([128, 512], F32R, tag="ef")
                                mh = mask_halves[mt // 4]
                                msl = mh[:, (mt % 4) * S + nq:(mt % 4) * S + nq + 512]
                                eng = nc.gpsimd if mt % 4 == 3 else nc.vector
                                eng.tensor_tensor(ef[:], ef0[:], msl, op=ALU.mult)
                                ef_tiles.append(ef)
                                nc.tensor.matmul(
                                    psav[:], vt_[:, mt * 128:(mt + 1) * 128], ef[:],
                                    start=(mt == 0), stop=(mt == 7))
                            for mt in range(8):
                                nc.tensor.matmul(psd[:], o128[:], ef_tiles[mt][:],
                                                 start=(mt == 0), stop=(mt == 7))
                            rd = epool.tile([1, 512], F32R, tag="rd")
                            nc.vector.reciprocal(rd[:], psd[:])
                            psbc = ps_d.tile([128, 512], F32, tag="dbc")
                            nc.tensor.matmul(psbc[:], o1[:], rd[:], start=True, stop=True)
                            bcs = epool.tile([128, 512], F32, tag="bcs")
                            nc.vector.tensor_copy(bcs[:], psbc[:])
                            nc.vector.tensor_tensor(at[:, nq:nq + 512], psav[:],
                                                    bcs[:], op=ALU.mult)
                        attn_t.append(at)

                    # W_o partial for batch b's tokens (half-row output tiles)
                    for m in range(8):
                        for half in range(4):
                            os_ = opool.tile([128, 1024], F32, tag="os")
                            for n in range(2):
                                nf = half * 1024 + n * 512
                                if b == B - 1 and (2 * half + n) % 2 == 1:
                                    pso = ps_av.tile([128, 512], F32, tag="av")
                                else:
                                    pso = ps_o.tile([128, 512], F32, tag="o")
                                for l in range(HC):
                                    nc.tensor.matmul(
                                        pso[:], attn_t[l][:, m * 128:(m + 1) * 128],
                                        wo_a[:, l * H + nf:l * H + nf + 512],
                                        start=(l == 0), stop=(l == HC - 1))
                                if n % 2 == 0:
                                    nc.vector.tensor_copy(
                                        os_[:, n * 512:(n + 1) * 512], pso[:])
                                else:
                                    nc.scalar.copy(
                                        os_[:, n * 512:(n + 1) * 512], pso[:])
                            nc.sync.dma_start(
                                out_p[bs + m * 128:bs + (m + 1) * 128,
                                      half * 1024:(half + 1) * 1024], os_[:])
    nc.compile()
    return nc


def _host_prep(hidden_states, W_pack, W_o, attention_mask, position_ids):
    import ml_dtypes
    hidden_states = np.asarray(hidden_states, dtype=np.float32)
    W_pack = np.asarray(W_pack, dtype=np.float32)
    W_o = np.asarray(W_o, dtype=np.float32)
    attention_mask = np.asarray(attention_mask, dtype=np.float32)
    pos = np.asarray(position_ids)

    xT = np.ascontiguousarray(hidden_states.reshape(T, H).T)
    # exp(mask): softmax mask applied multiplicatively after exp
    maskT = np.ascontiguousarray(
        np.exp(attention_mask[:, 0].transpose(0, 2, 1)).astype(ml_dtypes.bfloat16))

    inv = (1.0 / (ROPE_BASE ** (np.arange(0, D, 2, dtype=np.float64) / D)))
    inv = np.concatenate([inv, inv])                       # [D]
    ang = pos.astype(np.float64).reshape(T)[None, :] * inv[:, None]   # [D, T]
    cosT = np.cos(ang).astype(np.float32)
    sinT = np.sin(ang).astype(np.float32)
    sinS = sinT.copy()
    sinS[:64] = -sinT[:64]
    cosT = np.ascontiguousarray(cosT)
    sinS = np.ascontiguousarray(sinS)

    in_maps = []
    for c in range(N_CORES):
        h0 = c * HC
        # wqk column order per pass p: [q_{2p}, k_{2p}, q_{2p+1}, k_{2p+1}]
        qcols = [W_pack[:, (h0 + l) * D:(h0 + l + 1) * D] for l in range(HC)]
        kcols = [W_pack[:, H + (h0 + l) * D:H + (h0 + l + 1) * D] for l in range(HC)]
        vcols = [W_pack[:, 2 * H + (h0 + l) * D:2 * H + (h0 + l + 1) * D]
                 for l in range(HC)]
        wqk_np = np.ascontiguousarray(np.concatenate(
            [qcols[0], kcols[0], qcols[1], kcols[1],
             qcols[2], kcols[2], qcols[3], kcols[3]], axis=1))
        wv_np = np.ascontiguousarray(np.concatenate(vcols, axis=1))
        wo_np = np.ascontiguousarray(W_o[h0 * D:(h0 + HC) * D, :])
        in_maps.append({
            "xT": xT, "wqk": wqk_np, "wv": wv_np, "wo": wo_np,
            "cosT": cosT, "sinS": sinS, "maskT": maskT,
        })
    return in_maps


def kernel(hidden_states, W_pack, W_o, attention_mask, position_ids):
    if "nc" not in _CACHE:
        _CACHE["nc"] = _build_module()
    nc = _CACHE["nc"]
    in_maps = _host_prep(hidden_states, W_pack, W_o, attention_mask, position_ids)
    res = bass_utils.run_bass_kernel_spmd(nc, in_maps, core_ids=list(range(N_CORES)))
    out = res.results[0]["out_p"].astype(np.float64)
    for c in range(1, N_CORES):
        out += res.results[c]["out_p"]
    return out.reshape(B, S, H).astype(np.float32)


# revision 57
# speedup vs baseline: 1.3254x; 1.0089x over previous
"""Trainium2 Bass kernel for nn_Attention_60567628808865.

Dense transformer attention block (B=4, S=1024, H=4096, NH=32, D=128):
  qkv = x @ W_pack; RoPE(q, k); causal-masked softmax attention; out @ W_o.

Sharding: tensor-parallel over heads across 8 NeuronCores. Each core computes
4 heads end-to-end (QKV projection with its W_pack column slice, attention,
and its W_o row-slice partial of the output projection); the host sums the 8
partial outputs.

All matmuls run in float32r (TF32) at full PE rate; accumulation is fp32 in
PSUM. Everything on-chip works in a transposed layout (features on the
partition axis) so no transposes are needed anywhere:
  qT/kT [d, t] <- lhsT=W_qk, rhs=xT      scoresT [tk, tq] <- lhsT=kT, rhs=qT
  v [t, d]     <- lhsT=xT,   rhs=W_v     attnT [d, tq]    <- lhsT=v,  rhs=expT
  out [t, f]   <- lhsT=attnT, rhs=W_o
Softmax runs unnormalized (no max-subtraction; scores are O(1) by
construction and exp(-1e9)=0), with the denominator computed by a ones-vector
matmul accumulated in PSUM and applied after PV via a K=1 broadcast matmul.
RoPE's rotate-half is a partition shift, done for free in the DMA that loads
q/k back from scratch, with the sign folded into the host-built sin table.
DMAs are batched into multi-dim-AP transfers (the HWDGE issue path costs
~625ns per DMA instruction, so many small DMAs throttle the PE).
"""
import numpy as np

import concourse.bass as bass  # noqa: F401  (AP types come via tile/bacc)
import concourse.tile as tile
from contextlib import ExitStack
from concourse import bacc, mybir
from concourse import bass_utils

F32 = mybir.dt.float32
F32R = mybir.dt.float32r
BF16 = mybir.dt.bfloat16
AF = mybir.ActivationFunctionType
ALU = mybir.AluOpType

B, S, H, NH = 4, 1024, 4096, 32
D = H // NH          # 128
T = B * S            # 4096 tokens
N_CORES = 8
HC = NH // N_CORES   # 4 heads per core
SCALE = float(1.0 / np.sqrt(D))
ROPE_BASE = 10000.0

TB = 256             # phase-1 token block (moving dim; >=256 keeps f32r at full rate)
NTB = T // TB        # 16
KT = H // 128        # 32 contraction tiles

_CACHE = {}


def _build_module(phases=("p1", "p2")):
    nc = bacc.Bacc("TRN2", target_bir_lowering=False, debug=False,
                   num_devices=N_CORES)

    xT = nc.dram_tensor("xT", [H, T], F32R, kind="ExternalInput").ap()
    wqk = nc.dram_tensor("wqk", [H, 2 * HC * D], F32R, kind="ExternalInput").ap()
    wv = nc.dram_tensor("wv", [H, HC * D], F32R, kind="ExternalInput").ap()
    wo = nc.dram_tensor("wo", [HC * D, H], F32R, kind="ExternalInput").ap()
    cosT = nc.dram_tensor("cosT", [D, T], F32, kind="ExternalInput").ap()
    sinS = nc.dram_tensor("sinS", [D, T], F32, kind="ExternalInput").ap()
    maskT = nc.dram_tensor("maskT", [B, S, S], BF16, kind="ExternalInput").ap()
    out_p = nc.dram_tensor("out_p", [T, H], F32, kind="ExternalOutput").ap()
    ones128 = nc.inline_tensor(np.ones((128, 1), np.float32), "ones128").ap().bitcast(F32R)
    ones1 = nc.inline_tensor(np.ones((1, 128), np.float32), "ones1").ap().bitcast(F32R)

    with tile.TileContext(nc) as tc, \
         nc.allow_low_precision(reason="tf32 matmuls; verified against reference"):
        with ExitStack() as octx:
            dram = octx.enter_context(tc.tile_pool(name="dram", bufs=1, space="DRAM"))
            cpool = octx.enter_context(tc.tile_pool(name="consts", bufs=1))
            # scratch: qkT rows (pass p, m): [q_2p, k_2p, q_2p+1, k_2p+1]
            qkT_d = dram.tile([2 * HC * D, T], F32R)
            v_d = dram.tile([T, HC * D], F32R)

            o128 = cpool.tile([128, 1], F32R)
            nc.sync.dma_start(o128[:], ones128[:])
            o1 = cpool.tile([1, 128], F32R)
            nc.sync.dma_start(o1[:], ones1[:])

            # ---------------- Phase 1: QKV projection ----------------
            if "p1" in phases:
              with ExitStack() as ctx:
                wpool = ctx.enter_context(tc.tile_pool(name="p1w", bufs=1))
                xpool = ctx.enter_context(tc.tile_pool(name="p1x", bufs=2))
                opool = ctx.enter_context(tc.tile_pool(name="p1o", bufs=2))
                cpool1 = ctx.enter_context(tc.tile_pool(name="p1cs", bufs=2))
                rpool1 = ctx.enter_context(tc.tile_pool(name="p1rope", bufs=2))
                pqk = ctx.enter_context(tc.tile_pool(name="p1pqk", bufs=6, space="PSUM"))
                pv = ctx.enter_context(tc.tile_pool(name="p1pv", bufs=2, space="PSUM"))

                last_x = [None]

                def load_tb_inputs(p, tb):
                    t0 = tb * TB
                    cos_tb = cpool1.tile([128, TB], F32, tag="cos")
                    nc.sync.dma_start(cos_tb[:], cosT[:, t0:t0 + TB])
                    sin_tb = cpool1.tile([128, TB], F32, tag="sin")
                    nc.sync.dma_start(sin_tb[:], sinS[:, t0:t0 + TB])
                    if p == 1 and tb == NTB - 1 and last_x[0] is not None:
                        xall = last_x[0]
                    else:
                        xall = xpool.tile([128, KT * TB], F32R, tag="x")
                        for kh in range(2):
                            nc.sync.dma_start(
                                xall[:, kh * 16 * TB:(kh + 1) * 16 * TB]
                                    .rearrange("p (kk t) -> p kk t", kk=16),
                                xT[kh * 2048:(kh + 1) * 2048, t0:t0 + TB]
                                    .rearrange("(kk p) t -> p kk t", p=128))
                    if p == 0 and tb == NTB - 1:
                        last_x[0] = xall
                    return cos_tb, sin_tb, xall

                for p in range(2):
                    # resident weights, split into half-K DMAs so the first
                    # accumulation chains can start at half-load
                    first_inputs = load_tb_inputs(p, 0 if p == 0 else NTB - 1)
                    # weights as independent half-K tiles: pass p+1's low half
                    # can reload while pass p still reads the high half
                    KH = KT // 2
                    wqk_lo = wpool.tile([128, KH * 512], F32R, tag="wqk_lo")
                    wqk_hi = wpool.tile([128, KH * 512], F32R, tag="wqk_hi")
                    wv_lo = wpool.tile([128, KH * 256], F32R, tag="wv_lo")
                    wv_hi = wpool.tile([128, KH * 256], F32R, tag="wv_hi")
                    for kh, wt in ((0, wqk_lo), (1, wqk_hi)):
                        nc.sync.dma_start(
                            wt[:].rearrange("p (kk f) -> p kk f", kk=KH),
                            wqk[kh * KH * 128:(kh + 1) * KH * 128,
                                p * 512:(p + 1) * 512]
                                .rearrange("(kk p) f -> p kk f", p=128))
                    for kh, wt in ((0, wv_lo), (1, wv_hi)):
                        nc.sync.dma_start(
                            wt[:].rearrange("p (kk f) -> p kk f", kk=KH),
                            wv[kh * KH * 128:(kh + 1) * KH * 128,
                               p * 256:(p + 1) * 256]
                                .rearrange("(kk p) f -> p kk f", p=128))

                    def wqk_sl(kk, c0, c1):
                        wt = wqk_lo if kk < KH else wqk_hi
                        return wt[:, (kk % KH) * 512 + c0:(kk % KH) * 512 + c1]

                    def wv_sl(kk):
                        wt = wv_lo if kk < KH else wv_hi
                        return wt[:, (kk % KH) * 256:(kk % KH + 1) * 256]

                    tb_order = list(range(NTB)) if p == 0 else \
                        list(reversed(range(NTB)))
                    for tb in tb_order:
                        t0 = tb * TB
                        if tb == tb_order[0]:
                            cos_tb, sin_tb, xall = first_inputs
                        else:
                            cos_tb, sin_tb, xall = load_tb_inputs(p, tb)

                        qs_all = opool.tile([128, 4 * TB], F32R, tag="qs")
                        for m in range(4):
                            ps = pqk.tile([128, TB], F32, tag="qk")
                            for kk in range(KT):
                                nc.tensor.matmul(
                                    ps[:],
                                    wqk_sl(kk, m * 128, (m + 1) * 128),
                                    xall[:, kk * TB:(kk + 1) * TB],
                                    start=(kk == 0), stop=(kk == KT - 1))
                            # RoPE fused into the epilogue: rotate-half via
                            # partition-shifted copies, sign folded into sinS
                            rot = rpool1.tile([128, TB], F32, tag="rot")
                            nc.vector.tensor_copy(rot[0:64, :], ps[64:128, :])
                            nc.vector.tensor_copy(rot[64:128, :], ps[0:64, :])
                            m1_ = rpool1.tile([128, TB], F32, tag="m1")
                            nc.vector.tensor_tensor(m1_[:], ps[:], cos_tb[:],
                                                    op=ALU.mult)
                            m2_ = rpool1.tile([128, TB], F32, tag="m2")
                            nc.vector.tensor_tensor(m2_[:], rot[:], sin_tb[:],
                                                    op=ALU.mult)
                            nc.vector.tensor_tensor(qs_all[:, m * TB:(m + 1) * TB],
                                                    m1_[:], m2_[:], op=ALU.add)
                        # one DMA: rows (p*4+m)*128 for m in 0..4
                        nc.sync.dma_start(
                            qkT_d[p * 512:(p + 1) * 512, t0:t0 + TB]
                                .rearrange("(m pp) t -> pp m t", pp=128),
                            qs_all[:].rearrange("pp (m t) -> pp m t", m=4))

                        vs_all = opool.tile([128, 2 * 256], F32R, tag="vs")
                        for mt in range(2):
                            ps = pv.tile([128, 256], F32, tag="v")
                            for kk in range(KT):
                                nc.tensor.matmul(
                                    ps[:],
                                    xall[:, kk * TB + mt * 128:kk * TB + (mt + 1) * 128],
                                    wv_sl(kk),
                                    start=(kk == 0), stop=(kk == KT - 1))
                            nc.vector.tensor_copy(vs_all[:, mt * 256:(mt + 1) * 256], ps[:])
                        nc.sync.dma_start(
                            v_d[t0:t0 + TB, p * 256:(p + 1) * 256]
                                .rearrange("(mt pp) f -> pp mt f", pp=128),
                            vs_all[:].rearrange("pp (mt f) -> pp mt f", mt=2))

            # ---------------- Phase 2+3: attention + W_o ----------------
            if "p2" in phases:
              with ExitStack() as ctx:
                wopool = ctx.enter_context(tc.tile_pool(name="p2wo", bufs=1))
                mpool = ctx.enter_context(tc.tile_pool(name="p2m", bufs=1))
                m2pool = ctx.enter_context(tc.tile_pool(name="p2m2", bufs=2))
                tpool = ctx.enter_context(tc.tile_pool(name="p2t", bufs=2))
                epool = ctx.enter_context(tc.tile_pool(name="p2e", bufs=5))
                efpool = ctx.enter_context(tc.tile_pool(name="p2ef", bufs=10))
                apool = ctx.enter_context(tc.tile_pool(name="p2a", bufs=2))
                opool = ctx.enter_context(tc.tile_pool(name="p2o", bufs=3))
                ps_s = ctx.enter_context(tc.tile_pool(name="p2ps", bufs=3, space="PSUM"))
                ps_o = ctx.enter_context(tc.tile_pool(name="p2po", bufs=2, space="PSUM"))
                ps_d = ctx.enter_context(tc.tile_pool(name="p2pd", bufs=1, space="PSUM"))
                ps_av = ctx.enter_context(tc.tile_pool(name="p2pav", bufs=2, space="PSUM"))

                # W_o resident: one tile; DMA issued after the first head's
                # input loads so attention starts immediately
                wo_a = wopool.tile([128, HC * H], F32R, tag="wo")

                for b in range(B):
                    bs = b * S
                    mask_state = [None]

                    def load_mask():
                        mask_a = m2pool.tile([128, 4 * S], BF16, tag="maskA")
                        nc.sync.dma_start(
                            mask_a[:].rearrange("p (mt t) -> p mt t", mt=4),
                            maskT[b, 0:512].rearrange("(mt p) t -> p mt t", p=128))
                        mask_bb = mpool.tile([128, 4 * S], BF16, tag="maskB")
                        nc.sync.dma_start(
                            mask_bb[:].rearrange("p (mt t) -> p mt t", mt=4),
                            maskT[b, 512:1024].rearrange("(mt p) t -> p mt t", p=128))
                        mask_state[0] = (mask_a, mask_bb)

                    if b > 0:
                        load_mask()
                    attn_t = []
                    for l in range(HC):
                        rq = (4 * (l // 2) + 2 * (l % 2)) * 128
                        vcol = (l // 2) * 256 + (l % 2) * 128

                        # rope'd q,k load: [p, j(q/k), t] (1 DMA)
                        kq = tpool.tile([128, 2 * S], F32R, tag="kqraw")
                        nc.sync.dma_start(
                            kq[:].rearrange("p (j t) -> p j t", j=2),
                            qkT_d[rq:rq + 256, bs:bs + S]
                                .rearrange("(j p) t -> p j t", p=128))
                        vt_ = tpool.tile([128, 8 * 128], F32R, tag="vt")
                        nc.sync.dma_start(
                            vt_[:].rearrange("p (kt d) -> p kt d", kt=8),
                            v_d[bs:bs + S, vcol:vcol + 128]
                               .rearrange("(kt p) d -> p kt d", p=128))
                        q_rope = kq[:, 0:S]
                        k_rope = kq[:, S:2 * S]
                        if mask_state[0] is None:
                            load_mask()
                        mask_halves = mask_state[0]
                        if b == 0 and l >= 1:
                            lc = l - 1
                            nc.sync.dma_start(wo_a[:, lc * H:(lc + 1) * H],
                                              wo[lc * 128:(lc + 1) * 128, :])
                            if l == 3:
                                nc.sync.dma_start(wo_a[:, 3 * H:4 * H],
                                                  wo[3 * 128:4 * 128, :])

                        at = apool.tile([128, S], F32R, tag=f"attn{l}")
                        for nt in range(2):
                            nq = nt * 512
                            psd = ps_d.tile([1, 512], F32, tag="dbc")
                            psav = ps_av.tile([128, 512], F32, tag="av")
                            ef_tiles = []
                            for mt in range(8):
                                pss = ps_s.tile([128, 512], F32, tag="s")
                                nc.tensor.matmul(
                                    pss[:], k_rope[:, mt * 128:(mt + 1) * 128],
                                    q_rope[:, nq:nq + 512], start=True, stop=True)
                                ef0 = epool.tile([128, 512], F32, tag="ef0")
                                nc.scalar.activation(ef0[:], pss[:], AF.Exp,
                                                     scale=SCALE)
                                ef = efpool.tile([128, 512], F32R, tag="ef")
                                mh = mask_halves[mt // 4]
                                msl = mh[:, (mt % 4) * S + nq:(mt % 4) * S + nq + 512]
                                eng = nc.gpsimd if mt % 4 == 3 else nc.vector
                                eng.tensor_tensor(ef[:], ef0[:], msl, op=ALU.mult)
                                ef_tiles.append(ef)
                                nc.tensor.matmul(
                                    psav[:], vt_[:, mt * 128:(mt + 1) * 128], ef[:],
                                    start=(mt == 0), stop=(mt == 7))
                            for mt in range(8):
                                nc.tensor.matmul(psd[:], o128[:], ef_tiles[mt][:],
                                                 start=(mt == 0), stop=(mt == 7))
                            rd = epool.tile([1, 512], F32R, tag="rd")
                            nc.vector.reciprocal(rd[:], psd[:])
                            psbc = ps_d.tile([128, 512], F32, tag="dbc")
                            nc.tensor.matmul(psbc[:], o1[:], rd[:], start=True, stop=True)
                            bcs = epool.tile([128, 512], F32, tag="bcs")
                            nc.vector.tensor_copy(bcs[:], psbc[:])
                            nc.vector.tensor_tensor(at[:, nq:nq + 512], psav[:],
                                                    bcs[:], op=ALU.mult)
                        attn_t.append(at)

                    # W_o partial for batch b's tokens (half-row output tiles)
                    for m in range(8):
                        for half in range(4):
                            os_ = opool.tile([128, 1024], F32, tag="os")
                            for n in range(2):
                                nf = half * 1024 + n * 512
                                if b == B - 1 and (2 * half + n) % 2 == 1:
                                    pso = ps_av.tile([128, 512], F32, tag="av")
                                else:
                                    pso = ps_o.tile([128, 512], F32, tag="o")
                                for l in range(HC):
                                    nc.tensor.matmul(
                                        pso[:], attn_t[l][:, m * 128:(m + 1) * 128],
                                        wo_a[:, l * H + nf:l * H + nf + 512],
                                        start=(l == 0), stop=(l == HC - 1))
                                if n % 2 == 0:
                                    nc.vector.tensor_copy(
                                        os_[:, n * 512:(n + 1) * 512], pso[:])
                                else:
                                    nc.scalar.copy(
                                        os_[:, n * 512:(n + 1) * 512], pso[:])
                            nc.sync.dma_start(
                                out_p[bs + m * 128:bs + (m + 1) * 128,
                                      half * 1024:(half + 1) * 1024], os_[:])
    nc.compile()
    return nc


def _host_prep(hidden_states, W_pack, W_o, attention_mask, position_ids):
    import ml_dtypes
    hidden_states = np.asarray(hidden_states, dtype=np.float32)
    W_pack = np.asarray(W_pack, dtype=np.float32)
    W_o = np.asarray(W_o, dtype=np.float32)
    attention_mask = np.asarray(attention_mask, dtype=np.float32)
    pos = np.asarray(position_ids)

    xT = np.ascontiguousarray(hidden_states.reshape(T, H).T)
    # exp(mask): softmax mask applied multiplicatively after exp
    maskT = np.ascontiguousarray(
        np.exp(attention_mask[:, 0].transpose(0, 2, 1)).astype(ml_dtypes.bfloat16))

    inv = (1.0 / (ROPE_BASE ** (np.arange(0, D, 2, dtype=np.float64) / D)))
    inv = np.concatenate([inv, inv])                       # [D]
    ang = pos.astype(np.float64).reshape(T)[None, :] * inv[:, None]   # [D, T]
    cosT = np.cos(ang).astype(np.float32)
    sinT = np.sin(ang).astype(np.float32)
    sinS = sinT.copy()
    sinS[:64] = -sinT[:64]
    cosT = np.ascontiguousarray(cosT)
    sinS = np.ascontiguousarray(sinS)

    in_maps = []
    for c in range(N_CORES):
        h0 = c * HC
        # wqk column order per pass p: [q_{2p}, k_{2p}, q_{2p+1}, k_{2p+1}]
        qcols = [W_pack[:, (h0 + l) * D:(h0 + l + 1) * D] for l in range(HC)]
        kcols = [W_pack[:, H + (h0 + l) * D:H + (h0 + l + 1) * D] for l in range(HC)]
        vcols = [W_pack[:, 2 * H + (h0 + l) * D:2 * H + (h0 + l + 1) * D]
                 for l in range(HC)]
        wqk_np = np.ascontiguousarray(np.concatenate(
            [qcols[0], kcols[0], qcols[1], kcols[1],
             qcols[2], kcols[2], qcols[3], kcols[3]], axis=1))
        wv_np = np.ascontiguousarray(np.concatenate(vcols, axis=1))
        wo_np = np.ascontiguousarray(W_o[h0 * D:(h0 + HC) * D, :])
        in_maps.append({
            "xT": xT, "wqk": wqk_np, "wv": wv_np, "wo": wo_np,
            "cosT": cosT, "sinS": sinS, "maskT": maskT,
        })
    return in_maps


def kernel(hidden_states, W_pack, W_o, attention_mask, position_ids):
    if "nc" not in _CACHE:
        _CACHE["nc"] = _build_module()
    nc = _CACHE["nc"]
    in_maps = _host_prep(hidden_states, W_pack, W_o, attention_mask, position_ids)
    res = bass_utils.run_bass_kernel_spmd(nc, in_maps, core_ids=list(range(N_CORES)))
    out = res.results[0]["out_p"].astype(np.float64)
    for c in range(1, N_CORES):
        out += res.results[c]["out_p"]
    return out.reshape(B, S, H).astype(np.float32)
